# revision 3
# baseline (speedup 1.0000x reference)
"""Trainium2 Bass kernel for nn_AttenLayer (ragged-sequence attention pooling).

Math (per batch b, with length L_b):
    proj   = tanh(nn_outs @ W^T + b)           # (S, A)
    scores = proj @ context                     # (S,)
    atten  = masked_softmax(scores, L_b)        # (S,), zeros beyond L_b
    out    = atten @ nn_outs                    # (H,)

Ragged slot-capped data-parallel sharding over 8 cores:
  - All 64 batches are sorted by ceil(L/128) descending; rank window
    [8k, 8k+8) forms SLOT k, one batch per core per slot.  Every core
    runs the same instruction stream sized by the slot caps
    caps[k] = max ceil(L/128) over the window, so only ~Sigma caps*128
    tokens are computed/DMAd per core instead of 8*S.
  - Positions in [L_b, cap*128) are computed on real data but killed by
    the additive -30000 mask before softmax (exactly like full-width).
  - Waves of slots [0-3], [4-5], [6], [7] share a [wb, 512]-per-chunk
    scores PSUM via the zero-padded ctx trick; slot order descending
    guarantees the first writer covers the widest region (start=True).
    The two trailing single-slot waves keep the final (unoverlapped)
    softmax+phase-3 tail as small as possible.
  - ex (unnormalized exp) is transposed on PE and used directly as the
    phase-3 lhsT; the 1/sum normalization is applied to the final
    [1, H] output by the ACT copy (scale=rv), saving a [wb,S] pass.
"""

import sys

for _p in ("/opt/trn_rl_repo",):
    if _p not in sys.path:
        sys.path.insert(0, _p)

import numpy as np
import ml_dtypes

import concourse.bass as bass
from concourse import bacc
import concourse.mybir as mybir
import concourse.tile as tile
from concourse.masks import make_identity

B, S, H, A = 64, 2048, 512, 512
NCORES = 8
BPC = B // NCORES          # slots per core
WAVE_SPANS = [(0, 4), (4, 2), (6, 1), (7, 1)]
WB = 4                     # max wave size (ctx zero-pad layout width)

AC = A // 128              # 4 a-chunks
HC = H // 128              # 4 h-chunks

F32 = mybir.dt.float32
BF16 = mybir.dt.bfloat16


def build_nc(caps, widths, repeat: int = 1) -> bass.Bass:
    caps = [int(c) for c in caps]
    widths = [int(w) for w in widths]
    assert len(caps) == BPC and all(1 <= c <= S // 128 for c in caps)
    assert all(caps[i] >= caps[i + 1] for i in range(BPC - 1))
    assert all(widths[i] >= widths[i + 1] for i in range(BPC - 1))
    assert all((c - 1) * 128 < w <= c * 128 for c, w in zip(caps, widths))
    Wk = widths
    njs = [(w + 511) // 512 for w in Wk]
    xt_off = [0]
    nat_off = [0]
    for k in range(BPC):
        xt_off.append(xt_off[-1] + 4 * Wk[k])
        nat_off.append(nat_off[-1] + caps[k] * 512)

    nc = bacc.Bacc()

    xt_d = nc.declare_dram_parameter("xt", [128, xt_off[-1]], BF16, isOutput=False)
    nat_d = nc.declare_dram_parameter("nat", [128, nat_off[-1]], BF16, isOutput=False)
    # W^T pre-chunked on host: wt[p, c*A + a] = proj_w[a, 128c + p]
    wt_d = nc.declare_dram_parameter("wt", [128, HC * A], BF16, isOutput=False)
    ctx_d = nc.declare_dram_parameter("ctx", [128, AC * WB * WB], BF16, isOutput=False)
    pb_d = nc.declare_dram_parameter("pb", [128, AC], F32, isOutput=False)
    # mask rows regrouped per wave so each wave's rows start at partition 0
    mask_d = nc.declare_dram_parameter(
        "mask", [WB, len(WAVE_SPANS) * S], BF16, isOutput=False
    )
    out_d = nc.declare_dram_parameter("out", [BPC, H], F32, isOutput=True)

    with tile.TileContext(nc) as tc:
        with (
            tc.tile_pool(name="const", bufs=1) as const_pool,
            tc.tile_pool(name="xt", bufs=3) as xt_pool,
            tc.tile_pool(name="nat", bufs=5) as nat_pool,
            tc.tile_pool(name="projT", bufs=6) as proj_pool,
            tc.tile_pool(name="smx", bufs=2) as smx_pool,
            tc.tile_pool(name="attT", bufs=8) as attT_pool,
            tc.tile_pool(name="osb", bufs=4) as os_pool,
            tc.tile_pool(name="p1ps", bufs=2, space="PSUM") as p1_psum,
            tc.tile_pool(name="scps", bufs=4, space="PSUM") as sc_psum,
            tc.tile_pool(name="atps", bufs=1, space="PSUM") as at_psum,
            tc.tile_pool(name="ops", bufs=1, space="PSUM") as out_psum,
        ):
            # ---- constants (DMAs deferred to first use in slot 0) ----
            wt_sb = const_pool.tile([128, HC * A], BF16, tag="wt")
            ctx_sb = const_pool.tile([128, AC * WB * WB], BF16, tag="ctx")
            pb_sb = const_pool.tile([128, AC], F32, tag="pb")
            ident = const_pool.tile([WB, WB], BF16, tag="ident")
            make_identity(nc, ident[:])
            # natural tiles: fixed-size ring; at most 5 are live at once
            # (wave-0's four slots + the first slot of wave 1)
            nat_sb = {}

            mask_sb = const_pool.tile([WB, len(WAVE_SPANS) * S], BF16, tag="mask")
            scpss = {}  # w -> list of scores psum tiles [wb, 512]
            attT = {}   # (w, g) -> [128, 4*WB] bf16, col = wb*jj + bwi
            # last slot-in-wave that writes scores chunk j
            last_writer = {}
            for w, (b0, wb) in enumerate(WAVE_SPANS):
                last_writer[w] = [
                    max(bwi for bwi in range(wb) if njs[b0 + bwi] > j)
                    for j in range(njs[b0])
                ]

            def trace_slot(w, b0, wb, bwi):
                k = b0 + bwi
                W = Wk[k]
                nj = njs[k]
                xt = xt_pool.tile([128, 4, 2048], BF16, tag="xt")
                src = xt_d[:, xt_off[k] : xt_off[k] + 4 * W].rearrange(
                    "p (hc w) -> p hc w", hc=HC
                )
                if k == 0:
                    # interleave wt with the first xt chunks on the SP ring;
                    # the remaining xt pieces stream on the idle Pool ring so
                    # chunk j+1 lands while chunk j computes
                    c1 = min(W, 512)
                    for hc in range(HC):
                        nc.sync.dma_start(
                            wt_sb[:, hc * A : (hc + 1) * A],
                            wt_d[:, hc * A : (hc + 1) * A],
                        )
                        nc.sync.dma_start(xt[:, hc, :c1], src[:, hc, :c1])
                    nc.gpsimd.dma_start(pb_sb[:], pb_d[:])
                    nc.gpsimd.dma_start(ctx_sb[:], ctx_d[:])
                    nc.gpsimd.dma_start(mask_sb[:], mask_d[:])
                    for j in range(1, nj):
                        c0, c1 = j * 512, min(W, (j + 1) * 512)
                        eng = nc.gpsimd if j % 2 else nc.sync
                        eng.dma_start(xt[:, :, c0:c1], src[:, :, c0:c1])
                else:
                    nc.sync.dma_start(xt[:, :, :W], src[:])
                natk = nat_pool.tile([128, 16 * 512], BF16, tag="nat")
                nat_sb[k] = natk
                nc.gpsimd.dma_start(
                    natk[:, : caps[k] * 512],
                    nat_d[:, nat_off[k] : nat_off[k + 1]],
                )
                for j in range(nj):
                    wj = min(512, W - j * 512)
                    for a in range(AC):
                        ps = p1_psum.tile([128, 512], F32, tag="p1")
                        for hc in range(HC):
                            nc.tensor.matmul(
                                ps[:, :wj],
                                wt_sb[:, hc * A + a * 128 : hc * A + (a + 1) * 128],
                                xt[:, hc, j * 512 : j * 512 + wj],
                                start=(hc == 0),
                                stop=(hc == HC - 1),
                            )
                        pt = proj_pool.tile([128, 512], BF16, tag="projT")
                        nc.scalar.activation(
                            pt[:, :wj],
                            ps[:, :wj],
                            mybir.ActivationFunctionType.Tanh,
                            bias=pb_sb[:, a : a + 1],
                        )
                        # ctx col bwi is context's a-chunk, others zero, so only
                        # row bwi of the wave's scores psum accumulates slot k.
                        nc.tensor.matmul(
                            scpss[w][j][:, :wj],
                            ctx_sb[:, (a * WB + bwi) * WB : (a * WB + bwi) * WB + wb],
                            pt[:, :wj],
                            start=(bwi == 0 and a == 0),
                            stop=(bwi == last_writer[w][j] and a == AC - 1),
                        )

            def finish_wave(w, b0, wb):
                Wmax = Wk[b0]
                njw = njs[b0]
                capm = caps[b0]
                mw = mask_sb[:wb, w * S : (w + 1) * S]
                # masked scores: scm = scores + additive mask, chunkwise
                scm = smx_pool.tile([wb, S], F32, tag="scm")
                pmax = smx_pool.tile([wb, 4], F32, tag="pmax")
                for j in range(njw):
                    wjj = min(512, Wmax - j * 512)
                    sl = slice(j * 512, j * 512 + wjj)
                    nc.vector.tensor_tensor(
                        out=scm[:, sl], in0=scpss[w][j][:, :wjj],
                        in1=mw[:, sl], op=mybir.AluOpType.add,
                    )
                    nc.vector.reduce_max(
                        pmax[:, j : j + 1], scm[:, sl], axis=mybir.AxisListType.X
                    )
                mx = smx_pool.tile([wb, 1], F32, tag="mx")
                nc.vector.reduce_max(
                    mx[:], pmax[:, :njw], axis=mybir.AxisListType.X, negate=True
                )
                ex = smx_pool.tile([wb, S], BF16, tag="ex")
                rs = smx_pool.tile([wb, 1], F32, tag="rs")
                nc.scalar.activation(
                    ex[:, :Wmax],
                    scm[:, :Wmax],
                    mybir.ActivationFunctionType.Exp,
                    bias=mx[:],
                    accum_out=rs[:],
                )
                rv = smx_pool.tile([wb, 1], F32, tag="rv")
                nc.vector.reciprocal(rv[:], rs[:])
                # move rv to partition 0 (tiny SBUF->SBUF DMA) so the final
                # output copy can read it as a per-partition scale
                rvT = smx_pool.tile([1, WB], F32, tag="rvT")
                nc.gpsimd.dma_start(rvT[0:1, :wb], rv[:, 0:1])
                # zero the ex tail beyond the wave width so the last
                # 128-chunk transposes clean zeros
                if capm * 128 > Wmax:
                    nc.vector.memset(ex[:, Wmax : capm * 128], 0.0)
                # transpose ex chunks [wb, 128] -> [128, wb] on PE (bf16);
                # column stride padded to 2 so bf16 PSUM offsets stay 4B-aligned
                cs = wb if wb % 2 == 0 else wb + 1
                for g in range((capm + 3) // 4):
                    na = min(4, capm - 4 * g)
                    aps = at_psum.tile([128, 4 * WB], BF16, tag="atps")
                    for jj in range(na):
                        ch = 4 * g + jj
                        nc.tensor.transpose(
                            aps[:, jj * cs : jj * cs + wb],
                            ex[:, ch * 128 : (ch + 1) * 128],
                            ident[:wb, :wb],
                        )
                    att_sb = attT_pool.tile([128, 4 * WB], BF16, tag="attT")
                    nc.vector.tensor_copy(
                        att_sb[:, : na * cs].rearrange(
                            "p (n c) -> p n c", c=cs
                        )[:, :, :wb],
                        aps[:, : na * cs].rearrange(
                            "p (n c) -> p n c", c=cs
                        )[:, :, :wb],
                    )
                    attT[(w, g)] = att_sb
                # phase 3: out[k] = (sum_s ex[s] * x[s, :]) * rv
                # smallest slot first so its output DMA lands earliest
                for bwi in reversed(range(wb)):
                    k = b0 + bwi
                    ops = out_psum.tile([1, H], F32, tag="ops")
                    for n in range(caps[k]):
                        col = (n % 4) * cs + bwi
                        nc.tensor.matmul(
                            ops[:],
                            attT[(w, n // 4)][:, col : col + 1],
                            nat_sb[k][:, n * 512 : (n + 1) * 512],
                            start=(n == 0),
                            stop=(n == caps[k] - 1),
                        )
                    os_b = os_pool.tile([1, H], F32, tag="os")
                    nc.scalar.activation(
                        os_b[:],
                        ops[:],
                        mybir.ActivationFunctionType.Copy,
                        scale=rvT[0:1, bwi : bwi + 1],
                    )
                    nc.scalar.dma_start(out_d[k : k + 1, :], os_b[:])

            for _rep in range(repeat):
                scpss.clear()
                attT.clear()
                for w, (b0, wb) in enumerate(WAVE_SPANS):
                    scpss[w] = [
                        sc_psum.tile([wb, 512], F32, tag="scps", name="scps")
                        for _j in range(njs[b0])
                    ]
                    for bwi in range(wb):
                        trace_slot(w, b0, wb, bwi)
                        if bwi == 0 and w > 0:
                            pw = w - 1
                            finish_wave(pw, *WAVE_SPANS[pw])
                finish_wave(len(WAVE_SPANS) - 1, *WAVE_SPANS[-1])

    nc.finalize()
    return nc


_NC_CACHE = {}


def get_nc(caps, widths, repeat: int = 1) -> bass.Bass:
    key = (tuple(caps), tuple(widths), repeat)
    if key not in _NC_CACHE:
        _NC_CACHE[key] = build_nc(caps, widths, repeat=repeat)
    return _NC_CACHE[key]


def plan(lens):
    """Sort batches by length desc; slot k = ranks [8k, 8k+8), one per core.

    Window maxima of the descending sort minimize the summed per-slot caps
    (both the 20*L p1/scores term and the 512*ceil(L/128) p3 term are
    monotone in L, so length-sorting is optimal for the combined cost).
    """
    lens = np.asarray(lens).reshape(B).astype(np.int64)
    order = np.argsort(-lens, kind="stable")
    assign = order.reshape(BPC, NCORES)          # [slot, core] -> batch
    widths = [int(lens[assign[k, 0]]) for k in range(BPC)]
    caps = [(w + 127) // 128 for w in widths]
    return caps, widths, assign


def make_in_maps(nn_outs, batch_lens, context, proj_w, proj_b, caps, widths,
                 assign):
    x_bf = np.asarray(nn_outs, dtype=np.float32).astype(ml_dtypes.bfloat16)
    lens = np.asarray(batch_lens).reshape(B).astype(np.int64)
    wt = np.ascontiguousarray(np.asarray(proj_w, np.float32).T)  # [H, A]
    # wt_sb[p, c*A + a] = wt[128c + p, a]
    wt_host = np.ascontiguousarray(
        wt.reshape(HC, 128, A).transpose(1, 0, 2).reshape(128, HC * A)
    ).astype(ml_dtypes.bfloat16)
    ctx_c = np.asarray(context, np.float32).reshape(AC, 128)
    ctx_host = np.zeros((128, AC, WB, WB), np.float32)
    for a in range(AC):
        for bw in range(WB):
            ctx_host[:, a, bw, bw] = ctx_c[a]
    ctx_host = np.ascontiguousarray(
        ctx_host.reshape(128, AC * WB * WB)
    ).astype(ml_dtypes.bfloat16)
    pb_host = np.ascontiguousarray(
        np.asarray(proj_b, np.float32).reshape(AC, 128).T
    )
    iota = np.arange(S)[None, :]

    xt_w = sum(4 * w for w in widths)
    nat_w = sum(c * 512 for c in caps)
    in_maps = []
    for c in range(NCORES):
        xt_all = np.empty((128, xt_w), ml_dtypes.bfloat16)
        nat_all = np.empty((128, nat_w), ml_dtypes.bfloat16)
        mask = np.zeros((WB, len(WAVE_SPANS) * S), ml_dtypes.bfloat16)
        xo = no = 0
        for k in range(BPC):
            b = assign[k, c]
            W = widths[k]
            xa = x_bf[b, :W, :]                          # [W, H]
            xt_all[:, xo : xo + 4 * W] = (
                xa.T.reshape(HC, 128, W).transpose(1, 0, 2).reshape(128, 4 * W)
            )
            xo += 4 * W
            Wc = caps[k] * 128
            nat_all[:, no : no + caps[k] * 512] = (
                x_bf[b, :Wc, :].reshape(caps[k], 128, 512).transpose(1, 0, 2)
                .reshape(128, caps[k] * 512)
            )
            no += caps[k] * 512
        for w, (b0, wb) in enumerate(WAVE_SPANS):
            for bwi in range(wb):
                b = assign[b0 + bwi, c]
                mask[bwi, w * S : (w + 1) * S] = np.where(
                    iota[0] < lens[b], 0.0, -30000.0
                )
        in_maps.append(
            {
                "xt": xt_all,
                "nat": nat_all,
                "wt": wt_host,
                "ctx": ctx_host,
                "pb": pb_host,
                "mask": mask,
            }
        )
    return in_maps


def run(nn_outs, batch_lens, context, proj_w, proj_b, trace=False, repeat=1,
        **trace_kw):
    from concourse.bass_utils import run_bass_kernel_spmd

    caps, widths, assign = plan(batch_lens)
    nc = get_nc(caps, widths, repeat=repeat)
    in_maps = make_in_maps(
        nn_outs, batch_lens, context, proj_w, proj_b, caps, widths, assign
    )
    res = run_bass_kernel_spmd(
        nc, in_maps, list(range(NCORES)), trace=trace, **trace_kw
    )
    out = np.empty((B, H), np.float32)
    for c in range(NCORES):
        out[assign[:, c]] = res.results[c]["out"]
    return out, res


def kernel(nn_outs, batch_lens, context, proj_w, proj_b):
    out, _ = run(nn_outs, batch_lens, context, proj_w, proj_b, trace=False)
    return out


# revision 5
# speedup vs baseline: 1.0242x; 1.0242x over previous
"""Trainium2 Bass kernel for nn_AttenLayer (ragged-sequence attention pooling).

Math (per batch b, with length L_b):
    proj   = tanh(nn_outs @ W^T + b)           # (S, A)
    scores = proj @ context                     # (S,)
    atten  = masked_softmax(scores, L_b)        # (S,), zeros beyond L_b
    out    = atten @ nn_outs                    # (H,)

Ragged slot-capped data-parallel sharding over 8 cores:
  - All 64 batches are sorted by ceil(L/128) descending; rank window
    [8k, 8k+8) forms SLOT k, one batch per core per slot.  Every core
    runs the same instruction stream sized by the slot caps
    caps[k] = max ceil(L/128) over the window, so only ~Sigma caps*128
    tokens are computed/DMAd per core instead of 8*S.
  - Positions in [L_b, cap*128) are computed on real data but killed by
    the additive -30000 mask before softmax (exactly like full-width).
  - Waves of slots [0-3], [4-5], [6], [7] share a [wb, 512]-per-chunk
    scores PSUM via the zero-padded ctx trick; slot order descending
    guarantees the first writer covers the widest region (start=True).
    The two trailing single-slot waves keep the final (unoverlapped)
    softmax+phase-3 tail as small as possible.
  - ex (unnormalized exp) is transposed on PE and used directly as the
    phase-3 lhsT; the 1/sum normalization is applied to the final
    [1, H] output by the ACT copy (scale=rv), saving a [wb,S] pass.
"""

import sys

for _p in ("/opt/trn_rl_repo",):
    if _p not in sys.path:
        sys.path.insert(0, _p)

import numpy as np
import ml_dtypes

import concourse.bass as bass
from concourse import bacc
import concourse.mybir as mybir
import concourse.tile as tile
from concourse.masks import make_identity

B, S, H, A = 64, 2048, 512, 512
NCORES = 8
BPC = B // NCORES          # slots per core
WAVE_SPANS = [(0, 4), (4, 2), (6, 1), (7, 1)]
WB = 4                     # max wave size (ctx zero-pad layout width)

AC = A // 128              # 4 a-chunks
HC = H // 128              # 4 h-chunks

F32 = mybir.dt.float32
BF16 = mybir.dt.bfloat16


def build_nc(caps, widths, repeat: int = 1) -> bass.Bass:
    caps = [int(c) for c in caps]
    widths = [int(w) for w in widths]
    assert len(caps) == BPC and all(1 <= c <= S // 128 for c in caps)
    assert all(caps[i] >= caps[i + 1] for i in range(BPC - 1))
    assert all(widths[i] >= widths[i + 1] for i in range(BPC - 1))
    assert all((c - 1) * 128 < w <= c * 128 for c, w in zip(caps, widths))
    Wk = widths
    njs = [(w + 511) // 512 for w in Wk]
    xt_off = [0]
    nat_off = [0]
    for k in range(BPC):
        xt_off.append(xt_off[-1] + 4 * Wk[k])
        nat_off.append(nat_off[-1] + caps[k] * 512)

    nc = bacc.Bacc()

    xt_d = nc.declare_dram_parameter("xt", [128, xt_off[-1]], BF16, isOutput=False)
    nat_d = nc.declare_dram_parameter("nat", [128, nat_off[-1]], BF16, isOutput=False)
    # W^T pre-chunked on host: wt[p, c*A + a] = proj_w[a, 128c + p]
    wt_d = nc.declare_dram_parameter("wt", [128, HC * A], BF16, isOutput=False)
    ctx_d = nc.declare_dram_parameter("ctx", [128, AC * WB * WB], BF16, isOutput=False)
    pb_d = nc.declare_dram_parameter("pb", [128, AC], F32, isOutput=False)
    # mask rows regrouped per wave so each wave's rows start at partition 0
    mask_d = nc.declare_dram_parameter(
        "mask", [WB, len(WAVE_SPANS) * S], BF16, isOutput=False
    )
    out_d = nc.declare_dram_parameter("out", [BPC, H], F32, isOutput=True)

    with tile.TileContext(nc) as tc:
        with (
            tc.tile_pool(name="const", bufs=1) as const_pool,
            tc.tile_pool(name="xt", bufs=3) as xt_pool,
            tc.tile_pool(name="nat", bufs=5) as nat_pool,
            tc.tile_pool(name="projT", bufs=10) as proj_pool,
            tc.tile_pool(name="smx", bufs=2) as smx_pool,
            tc.tile_pool(name="attT", bufs=8) as attT_pool,
            tc.tile_pool(name="osb", bufs=4) as os_pool,
            tc.tile_pool(name="p1ps", bufs=2, space="PSUM") as p1_psum,
            tc.tile_pool(name="scps", bufs=4, space="PSUM") as sc_psum,
            tc.tile_pool(name="atps", bufs=1, space="PSUM") as at_psum,
            tc.tile_pool(name="ops", bufs=1, space="PSUM") as out_psum,
        ):
            # ---- constants (DMAs deferred to first use in slot 0) ----
            wt_sb = const_pool.tile([128, HC * A], BF16, tag="wt")
            ctx_sb = const_pool.tile([128, AC * WB * WB], BF16, tag="ctx")
            pb_sb = const_pool.tile([128, AC], F32, tag="pb")
            ident = const_pool.tile([WB, WB], BF16, tag="ident")
            make_identity(nc, ident[:])
            # natural tiles: fixed-size ring; at most 5 are live at once
            # (wave-0's four slots + the first slot of wave 1)
            nat_sb = {}

            mask_sb = const_pool.tile([WB, len(WAVE_SPANS) * S], BF16, tag="mask")
            scpss = {}  # w -> list of scores psum tiles [wb, 512]
            attT = {}   # (w, g) -> [128, 4*WB] bf16, col = wb*jj + bwi
            # last slot-in-wave that writes scores chunk j
            last_writer = {}
            for w, (b0, wb) in enumerate(WAVE_SPANS):
                last_writer[w] = [
                    max(bwi for bwi in range(wb) if njs[b0 + bwi] > j)
                    for j in range(njs[b0])
                ]

            def trace_slot(w, b0, wb, bwi):
                k = b0 + bwi
                W = Wk[k]
                nj = njs[k]
                xt = xt_pool.tile([128, 4, 2048], BF16, tag="xt")
                src = xt_d[:, xt_off[k] : xt_off[k] + 4 * W].rearrange(
                    "p (hc w) -> p hc w", hc=HC
                )
                if k == 0:
                    # interleave wt with the first xt chunks on the SP ring;
                    # the remaining xt pieces stream on the idle Pool ring so
                    # chunk j+1 lands while chunk j computes
                    c1 = min(W, 512)
                    for hc in range(HC):
                        nc.sync.dma_start(
                            wt_sb[:, hc * A : (hc + 1) * A],
                            wt_d[:, hc * A : (hc + 1) * A],
                        )
                        nc.sync.dma_start(xt[:, hc, :c1], src[:, hc, :c1])
                    nc.gpsimd.dma_start(pb_sb[:], pb_d[:])
                    nc.gpsimd.dma_start(ctx_sb[:], ctx_d[:])
                    for j in range(1, nj):
                        c0, c1 = j * 512, min(W, (j + 1) * 512)
                        eng = nc.gpsimd if j % 2 else nc.sync
                        eng.dma_start(xt[:, :, c0:c1], src[:, :, c0:c1])
                    # mask is not needed until the first softmax; keep it
                    # behind the latency-critical xt pieces
                    nc.gpsimd.dma_start(mask_sb[:], mask_d[:])
                else:
                    nc.sync.dma_start(xt[:, :, :W], src[:])
                natk = nat_pool.tile([128, 16 * 512], BF16, tag="nat")
                nat_sb[k] = natk
                nc.gpsimd.dma_start(
                    natk[:, : caps[k] * 512],
                    nat_d[:, nat_off[k] : nat_off[k + 1]],
                )
                pq = []  # software-pipeline: scores MMs trail p1 by 8 a-steps
                for j in range(nj):
                    wj = min(512, W - j * 512)
                    for a in range(AC):
                        ps = p1_psum.tile([128, 512], F32, tag="p1")
                        for hc in range(HC):
                            nc.tensor.matmul(
                                ps[:, :wj],
                                wt_sb[:, hc * A + a * 128 : hc * A + (a + 1) * 128],
                                xt[:, hc, j * 512 : j * 512 + wj],
                                start=(hc == 0),
                                stop=(hc == HC - 1),
                            )
                        if len(pq) >= 8:
                            pq.pop(0)()
                        pt = proj_pool.tile([128, 512], BF16, tag="projT")
                        nc.scalar.activation(
                            pt[:, :wj],
                            ps[:, :wj],
                            mybir.ActivationFunctionType.Tanh,
                            bias=pb_sb[:, a : a + 1],
                        )
                        # ctx col bwi is context's a-chunk, others zero, so only
                        # row bwi of the wave's scores psum accumulates slot k.
                        def pend(pt=pt, j=j, a=a, wj=wj):
                            nc.tensor.matmul(
                                scpss[w][j][:, :wj],
                                ctx_sb[:, (a * WB + bwi) * WB : (a * WB + bwi) * WB + wb],
                                pt[:, :wj],
                                start=(bwi == 0 and a == 0),
                                stop=(bwi == last_writer[w][j] and a == AC - 1),
                            )
                        pq.append(pend)
                for f in pq:
                    f()

            def finish_wave(w, b0, wb):
                Wmax = Wk[b0]
                njw = njs[b0]
                capm = caps[b0]
                mw = mask_sb[:wb, w * S : (w + 1) * S]
                # masked scores: scm = scores + additive mask, chunkwise
                scm = smx_pool.tile([wb, S], F32, tag="scm")
                pmax = smx_pool.tile([wb, 4], F32, tag="pmax")
                for j in range(njw):
                    wjj = min(512, Wmax - j * 512)
                    sl = slice(j * 512, j * 512 + wjj)
                    nc.vector.tensor_tensor(
                        out=scm[:, sl], in0=scpss[w][j][:, :wjj],
                        in1=mw[:, sl], op=mybir.AluOpType.add,
                    )
                    nc.vector.reduce_max(
                        pmax[:, j : j + 1], scm[:, sl], axis=mybir.AxisListType.X
                    )
                mx = smx_pool.tile([wb, 1], F32, tag="mx")
                nc.vector.reduce_max(
                    mx[:], pmax[:, :njw], axis=mybir.AxisListType.X, negate=True
                )
                ex = smx_pool.tile([wb, S], BF16, tag="ex")
                rs = smx_pool.tile([wb, 1], F32, tag="rs")
                nc.scalar.activation(
                    ex[:, :Wmax],
                    scm[:, :Wmax],
                    mybir.ActivationFunctionType.Exp,
                    bias=mx[:],
                    accum_out=rs[:],
                )
                rv = smx_pool.tile([wb, 1], F32, tag="rv")
                nc.vector.reciprocal(rv[:], rs[:])
                # move rv to partition 0 (tiny SBUF->SBUF DMA) so the final
                # output copy can read it as a per-partition scale
                rvT = smx_pool.tile([1, WB], F32, tag="rvT")
                nc.gpsimd.dma_start(rvT[0:1, :wb], rv[:, 0:1])
                # zero the ex tail beyond the wave width so the last
                # 128-chunk transposes clean zeros
                if capm * 128 > Wmax:
                    nc.vector.memset(ex[:, Wmax : capm * 128], 0.0)
                # transpose ex chunks [wb, 128] -> [128, wb] on PE (bf16);
                # column stride padded to 2 so bf16 PSUM offsets stay 4B-aligned
                cs = wb if wb % 2 == 0 else wb + 1
                for g in range((capm + 3) // 4):
                    na = min(4, capm - 4 * g)
                    aps = at_psum.tile([128, 4 * WB], BF16, tag="atps")
                    for jj in range(na):
                        ch = 4 * g + jj
                        nc.tensor.transpose(
                            aps[:, jj * cs : jj * cs + wb],
                            ex[:, ch * 128 : (ch + 1) * 128],
                            ident[:wb, :wb],
                        )
                    att_sb = attT_pool.tile([128, 4 * WB], BF16, tag="attT")
                    nc.vector.tensor_copy(
                        att_sb[:, : na * cs].rearrange(
                            "p (n c) -> p n c", c=cs
                        )[:, :, :wb],
                        aps[:, : na * cs].rearrange(
                            "p (n c) -> p n c", c=cs
                        )[:, :, :wb],
                    )
                    attT[(w, g)] = att_sb
                # phase 3: out[k] = (sum_s ex[s] * x[s, :]) * rv
                # smallest slot first so its output DMA lands earliest
                for bwi in reversed(range(wb)):
                    k = b0 + bwi
                    ops = out_psum.tile([1, H], F32, tag="ops")
                    for n in range(caps[k]):
                        col = (n % 4) * cs + bwi
                        nc.tensor.matmul(
                            ops[:],
                            attT[(w, n // 4)][:, col : col + 1],
                            nat_sb[k][:, n * 512 : (n + 1) * 512],
                            start=(n == 0),
                            stop=(n == caps[k] - 1),
                        )
                    os_b = os_pool.tile([1, H], F32, tag="os")
                    nc.scalar.activation(
                        os_b[:],
                        ops[:],
                        mybir.ActivationFunctionType.Copy,
                        scale=rvT[0:1, bwi : bwi + 1],
                    )
                    nc.scalar.dma_start(out_d[k : k + 1, :], os_b[:])

            for _rep in range(repeat):
                scpss.clear()
                attT.clear()
                for w, (b0, wb) in enumerate(WAVE_SPANS):
                    scpss[w] = [
                        sc_psum.tile([wb, 512], F32, tag="scps", name="scps")
                        for _j in range(njs[b0])
                    ]
                    for bwi in range(wb):
                        trace_slot(w, b0, wb, bwi)
                        if bwi == 0 and w > 0:
                            pw = w - 1
                            finish_wave(pw, *WAVE_SPANS[pw])
                finish_wave(len(WAVE_SPANS) - 1, *WAVE_SPANS[-1])

    nc.finalize()
    return nc


_NC_CACHE = {}


def get_nc(caps, widths, repeat: int = 1) -> bass.Bass:
    key = (tuple(caps), tuple(widths), repeat)
    if key not in _NC_CACHE:
        _NC_CACHE[key] = build_nc(caps, widths, repeat=repeat)
    return _NC_CACHE[key]


def plan(lens):
    """Sort batches by length desc; slot k = ranks [8k, 8k+8), one per core.

    Window maxima of the descending sort minimize the summed per-slot caps
    (both the 20*L p1/scores term and the 512*ceil(L/128) p3 term are
    monotone in L, so length-sorting is optimal for the combined cost).
    """
    lens = np.asarray(lens).reshape(B).astype(np.int64)
    order = np.argsort(-lens, kind="stable")
    assign = order.reshape(BPC, NCORES)          # [slot, core] -> batch
    widths = [int(lens[assign[k, 0]]) for k in range(BPC)]
    caps = [(w + 127) // 128 for w in widths]
    return caps, widths, assign


def make_in_maps(nn_outs, batch_lens, context, proj_w, proj_b, caps, widths,
                 assign):
    x_bf = np.asarray(nn_outs, dtype=np.float32).astype(ml_dtypes.bfloat16)
    lens = np.asarray(batch_lens).reshape(B).astype(np.int64)
    wt = np.ascontiguousarray(np.asarray(proj_w, np.float32).T)  # [H, A]
    # wt_sb[p, c*A + a] = wt[128c + p, a]
    wt_host = np.ascontiguousarray(
        wt.reshape(HC, 128, A).transpose(1, 0, 2).reshape(128, HC * A)
    ).astype(ml_dtypes.bfloat16)
    ctx_c = np.asarray(context, np.float32).reshape(AC, 128)
    ctx_host = np.zeros((128, AC, WB, WB), np.float32)
    for a in range(AC):
        for bw in range(WB):
            ctx_host[:, a, bw, bw] = ctx_c[a]
    ctx_host = np.ascontiguousarray(
        ctx_host.reshape(128, AC * WB * WB)
    ).astype(ml_dtypes.bfloat16)
    pb_host = np.ascontiguousarray(
        np.asarray(proj_b, np.float32).reshape(AC, 128).T
    )
    iota = np.arange(S)[None, :]

    xt_w = sum(4 * w for w in widths)
    nat_w = sum(c * 512 for c in caps)
    in_maps = []
    for c in range(NCORES):
        xt_all = np.empty((128, xt_w), ml_dtypes.bfloat16)
        nat_all = np.empty((128, nat_w), ml_dtypes.bfloat16)
        mask = np.zeros((WB, len(WAVE_SPANS) * S), ml_dtypes.bfloat16)
        xo = no = 0
        for k in range(BPC):
            b = assign[k, c]
            W = widths[k]
            xa = x_bf[b, :W, :]                          # [W, H]
            xt_all[:, xo : xo + 4 * W] = (
                xa.T.reshape(HC, 128, W).transpose(1, 0, 2).reshape(128, 4 * W)
            )
            xo += 4 * W
            Wc = caps[k] * 128
            nat_all[:, no : no + caps[k] * 512] = (
                x_bf[b, :Wc, :].reshape(caps[k], 128, 512).transpose(1, 0, 2)
                .reshape(128, caps[k] * 512)
            )
            no += caps[k] * 512
        for w, (b0, wb) in enumerate(WAVE_SPANS):
            for bwi in range(wb):
                b = assign[b0 + bwi, c]
                mask[bwi, w * S : (w + 1) * S] = np.where(
                    iota[0] < lens[b], 0.0, -30000.0
                )
        in_maps.append(
            {
                "xt": xt_all,
                "nat": nat_all,
                "wt": wt_host,
                "ctx": ctx_host,
                "pb": pb_host,
                "mask": mask,
            }
        )
    return in_maps


def run(nn_outs, batch_lens, context, proj_w, proj_b, trace=False, repeat=1,
        **trace_kw):
    from concourse.bass_utils import run_bass_kernel_spmd

    caps, widths, assign = plan(batch_lens)
    nc = get_nc(caps, widths, repeat=repeat)
    in_maps = make_in_maps(
        nn_outs, batch_lens, context, proj_w, proj_b, caps, widths, assign
    )
    res = run_bass_kernel_spmd(
        nc, in_maps, list(range(NCORES)), trace=trace, **trace_kw
    )
    out = np.empty((B, H), np.float32)
    for c in range(NCORES):
        out[assign[:, c]] = res.results[c]["out"]
    return out, res


def kernel(nn_outs, batch_lens, context, proj_w, proj_b):
    out, _ = run(nn_outs, batch_lens, context, proj_w, proj_b, trace=False)
    return out


# revision 6
# speedup vs baseline: 1.0245x; 1.0003x over previous
"""Trainium2 Bass kernel for nn_AttenLayer (ragged-sequence attention pooling).

Math (per batch b, with length L_b):
    proj   = tanh(nn_outs @ W^T + b)           # (S, A)
    scores = proj @ context                     # (S,)
    atten  = masked_softmax(scores, L_b)        # (S,), zeros beyond L_b
    out    = atten @ nn_outs                    # (H,)

Ragged slot-capped data-parallel sharding over 8 cores:
  - All 64 batches are sorted by ceil(L/128) descending; rank window
    [8k, 8k+8) forms SLOT k, one batch per core per slot.  Every core
    runs the same instruction stream sized by the slot caps
    caps[k] = max ceil(L/128) over the window, so only ~Sigma caps*128
    tokens are computed/DMAd per core instead of 8*S.
  - Positions in [L_b, cap*128) are computed on real data but killed by
    the additive -30000 mask before softmax (exactly like full-width).
  - Waves of slots [0-3], [4-5], [6], [7] share a [wb, 512]-per-chunk
    scores PSUM via the zero-padded ctx trick; slot order descending
    guarantees the first writer covers the widest region (start=True).
    The two trailing single-slot waves keep the final (unoverlapped)
    softmax+phase-3 tail as small as possible.
  - ex (unnormalized exp) is transposed on PE and used directly as the
    phase-3 lhsT; the 1/sum normalization is applied to the final
    [1, H] output by the ACT copy (scale=rv), saving a [wb,S] pass.
"""

import sys

for _p in ("/opt/trn_rl_repo",):
    if _p not in sys.path:
        sys.path.insert(0, _p)

import numpy as np
import ml_dtypes

import concourse.bass as bass
from concourse import bacc
import concourse.mybir as mybir
import concourse.tile as tile
from concourse.masks import make_identity

B, S, H, A = 64, 2048, 512, 512
NCORES = 8
BPC = B // NCORES          # slots per core
WAVE_SPANS = [(0, 4), (4, 2), (6, 1), (7, 1)]
WB = 4                     # max wave size (ctx zero-pad layout width)

AC = A // 128              # 4 a-chunks
HC = H // 128              # 4 h-chunks

F32 = mybir.dt.float32
BF16 = mybir.dt.bfloat16


def build_nc(caps, widths, repeat: int = 1) -> bass.Bass:
    caps = [int(c) for c in caps]
    widths = [int(w) for w in widths]
    assert len(caps) == BPC and all(1 <= c <= S // 128 for c in caps)
    assert all(caps[i] >= caps[i + 1] for i in range(BPC - 1))
    assert all(widths[i] >= widths[i + 1] for i in range(BPC - 1))
    assert all((c - 1) * 128 < w <= c * 128 for c, w in zip(caps, widths))
    Wk = widths
    njs = [(w + 511) // 512 for w in Wk]
    xt_off = [0]
    nat_off = [0]
    for k in range(BPC):
        xt_off.append(xt_off[-1] + 4 * Wk[k])
        nat_off.append(nat_off[-1] + caps[k] * 512)

    nc = bacc.Bacc()

    xt_d = nc.declare_dram_parameter("xt", [128, xt_off[-1]], BF16, isOutput=False)
    nat_d = nc.declare_dram_parameter("nat", [128, nat_off[-1]], BF16, isOutput=False)
    # W^T pre-chunked on host: wt[p, c*A + a] = proj_w[a, 128c + p]
    wt_d = nc.declare_dram_parameter("wt", [128, HC * A], BF16, isOutput=False)
    ctx_d = nc.declare_dram_parameter("ctx", [128, AC * WB * WB], BF16, isOutput=False)
    pb_d = nc.declare_dram_parameter("pb", [128, AC], F32, isOutput=False)
    # mask rows regrouped per wave so each wave's rows start at partition 0
    mask_d = nc.declare_dram_parameter(
        "mask", [WB, len(WAVE_SPANS) * S], BF16, isOutput=False
    )
    out_d = nc.declare_dram_parameter("out", [BPC, H], F32, isOutput=True)

    with tile.TileContext(nc) as tc:
        with (
            tc.tile_pool(name="const", bufs=1) as const_pool,
            tc.tile_pool(name="xt", bufs=3) as xt_pool,
            tc.tile_pool(name="nat", bufs=5) as nat_pool,
            tc.tile_pool(name="projT", bufs=10) as proj_pool,
            tc.tile_pool(name="smx", bufs=2) as smx_pool,
            tc.tile_pool(name="attT", bufs=8) as attT_pool,
            tc.tile_pool(name="osb", bufs=4) as os_pool,
            tc.tile_pool(name="p1ps", bufs=2, space="PSUM") as p1_psum,
            tc.tile_pool(name="scps", bufs=4, space="PSUM") as sc_psum,
            tc.tile_pool(name="atps", bufs=1, space="PSUM") as at_psum,
            tc.tile_pool(name="ops", bufs=1, space="PSUM") as out_psum,
        ):
            # ---- constants (DMAs deferred to first use in slot 0) ----
            wt_sb = const_pool.tile([128, HC * A], BF16, tag="wt")
            ctx_sb = const_pool.tile([128, AC * WB * WB], BF16, tag="ctx")
            pb_sb = const_pool.tile([128, AC], F32, tag="pb")
            ident = const_pool.tile([WB, WB], BF16, tag="ident")
            make_identity(nc, ident[:])
            # natural tiles: fixed-size ring; at most 5 are live at once
            # (wave-0's four slots + the first slot of wave 1)
            nat_sb = {}

            mask_sb = const_pool.tile([WB, len(WAVE_SPANS) * S], BF16, tag="mask")
            scpss = {}  # w -> list of scores psum tiles [wb, 512]
            attT = {}   # (w, g) -> [128, 4*WB] bf16, col = wb*jj + bwi
            # last slot-in-wave that writes scores chunk j
            last_writer = {}
            for w, (b0, wb) in enumerate(WAVE_SPANS):
                last_writer[w] = [
                    max(bwi for bwi in range(wb) if njs[b0 + bwi] > j)
                    for j in range(njs[b0])
                ]

            def trace_slot(w, b0, wb, bwi):
                k = b0 + bwi
                W = Wk[k]
                nj = njs[k]
                xt = xt_pool.tile([128, 4, 2048], BF16, tag="xt")
                src = xt_d[:, xt_off[k] : xt_off[k] + 4 * W].rearrange(
                    "p (hc w) -> p hc w", hc=HC
                )
                if k == 0:
                    # interleave wt with the first xt chunks on the SP ring;
                    # the remaining xt pieces stream on the idle Pool ring so
                    # chunk j+1 lands while chunk j computes
                    c1 = min(W, 512)
                    for hc in range(HC):
                        nc.sync.dma_start(
                            wt_sb[:, hc * A : (hc + 1) * A],
                            wt_d[:, hc * A : (hc + 1) * A],
                        )
                        nc.sync.dma_start(xt[:, hc, :c1], src[:, hc, :c1])
                    nc.gpsimd.dma_start(pb_sb[:], pb_d[:])
                    nc.gpsimd.dma_start(ctx_sb[:], ctx_d[:])
                    for j in range(1, nj):
                        c0, c1 = j * 512, min(W, (j + 1) * 512)
                        eng = nc.gpsimd if j % 2 else nc.sync
                        eng.dma_start(xt[:, :, c0:c1], src[:, :, c0:c1])
                    # mask is not needed until the first softmax; keep it
                    # behind the latency-critical xt pieces
                    nc.gpsimd.dma_start(mask_sb[:], mask_d[:])
                else:
                    nc.sync.dma_start(xt[:, :, :W], src[:])
                natk = nat_pool.tile([128, 16 * 512], BF16, tag="nat")
                nat_sb[k] = natk
                nc.gpsimd.dma_start(
                    natk[:, : caps[k] * 512],
                    nat_d[:, nat_off[k] : nat_off[k + 1]],
                )
                pq = []  # software-pipeline: scores MMs trail p1 by 8 a-steps
                for j in range(nj):
                    wj = min(512, W - j * 512)
                    for a in range(AC):
                        ps = p1_psum.tile([128, 512], F32, tag="p1")
                        for hc in range(HC):
                            nc.tensor.matmul(
                                ps[:, :wj],
                                wt_sb[:, hc * A + a * 128 : hc * A + (a + 1) * 128],
                                xt[:, hc, j * 512 : j * 512 + wj],
                                start=(hc == 0),
                                stop=(hc == HC - 1),
                            )
                        if len(pq) >= 8:
                            pq.pop(0)()
                        pt = proj_pool.tile([128, 512], BF16, tag="projT")
                        nc.scalar.activation(
                            pt[:, :wj],
                            ps[:, :wj],
                            mybir.ActivationFunctionType.Tanh,
                            bias=pb_sb[:, a : a + 1],
                        )
                        # ctx col bwi is context's a-chunk, others zero, so only
                        # row bwi of the wave's scores psum accumulates slot k.
                        def pend(pt=pt, j=j, a=a, wj=wj):
                            nc.tensor.matmul(
                                scpss[w][j][:, :wj],
                                ctx_sb[:, (a * WB + bwi) * WB : (a * WB + bwi) * WB + wb],
                                pt[:, :wj],
                                start=(bwi == 0 and a == 0),
                                stop=(bwi == last_writer[w][j] and a == AC - 1),
                            )
                        pq.append(pend)
                for f in pq:
                    f()

            def finish_wave(w, b0, wb):
                Wmax = Wk[b0]
                njw = njs[b0]
                capm = caps[b0]
                mw = mask_sb[:wb, w * S : (w + 1) * S]
                # masked scores: scm = scores + additive mask, chunkwise
                scm = smx_pool.tile([wb, S], F32, tag="scm")
                pmax = smx_pool.tile([wb, 4], F32, tag="pmax")
                for j in range(njw):
                    wjj = min(512, Wmax - j * 512)
                    sl = slice(j * 512, j * 512 + wjj)
                    nc.vector.tensor_tensor(
                        out=scm[:, sl], in0=scpss[w][j][:, :wjj],
                        in1=mw[:, sl], op=mybir.AluOpType.add,
                    )
                    nc.vector.reduce_max(
                        pmax[:, j : j + 1], scm[:, sl], axis=mybir.AxisListType.X
                    )
                mx = smx_pool.tile([wb, 1], F32, tag="mx")
                nc.vector.reduce_max(
                    mx[:], pmax[:, :njw], axis=mybir.AxisListType.X, negate=True
                )
                ex = smx_pool.tile([wb, S], BF16, tag="ex")
                rs = smx_pool.tile([wb, 1], F32, tag="rs")
                nc.scalar.activation(
                    ex[:, :Wmax],
                    scm[:, :Wmax],
                    mybir.ActivationFunctionType.Exp,
                    bias=mx[:],
                    accum_out=rs[:],
                )
                rv = smx_pool.tile([wb, 1], F32, tag="rv")
                nc.vector.reciprocal(rv[:], rs[:])
                # move rv to partition 0 (tiny SBUF->SBUF DMA) so the final
                # output copy can read it as a per-partition scale
                rvT = smx_pool.tile([1, WB], F32, tag="rvT")
                nc.gpsimd.dma_start(rvT[0:1, :wb], rv[:, 0:1])
                # zero the ex tail beyond the wave width so the last
                # 128-chunk transposes clean zeros
                if capm * 128 > Wmax:
                    nc.vector.memset(ex[:, Wmax : capm * 128], 0.0)
                # transpose ex chunks [wb, 128] -> [128, wb] on PE (bf16);
                # column stride padded to 2 so bf16 PSUM offsets stay 4B-aligned
                cs = wb if wb % 2 == 0 else wb + 1
                for g in range((capm + 3) // 4):
                    na = min(4, capm - 4 * g)
                    aps = at_psum.tile([128, 4 * WB], BF16, tag="atps")
                    for jj in range(na):
                        ch = 4 * g + jj
                        nc.tensor.transpose(
                            aps[:, jj * cs : jj * cs + wb],
                            ex[:, ch * 128 : (ch + 1) * 128],
                            ident[:wb, :wb],
                        )
                    att_sb = attT_pool.tile([128, 4 * WB], BF16, tag="attT")
                    nc.vector.tensor_copy(
                        att_sb[:, : na * cs].rearrange(
                            "p (n c) -> p n c", c=cs
                        )[:, :, :wb],
                        aps[:, : na * cs].rearrange(
                            "p (n c) -> p n c", c=cs
                        )[:, :, :wb],
                    )
                    attT[(w, g)] = att_sb
                # phase 3: out[k] = (sum_s ex[s] * x[s, :]) * rv
                # smallest slot first so its output DMA lands earliest
                for bwi in reversed(range(wb)):
                    k = b0 + bwi
                    ops = out_psum.tile([1, H], F32, tag="ops")
                    for n in range(caps[k]):
                        col = (n % 4) * cs + bwi
                        nc.tensor.matmul(
                            ops[:],
                            attT[(w, n // 4)][:, col : col + 1],
                            nat_sb[k][:, n * 512 : (n + 1) * 512],
                            start=(n == 0),
                            stop=(n == caps[k] - 1),
                        )
                    os_b = os_pool.tile([1, H], F32, tag="os")
                    nc.vector.tensor_scalar_mul(
                        os_b[:], ops[:], rvT[0:1, bwi : bwi + 1]
                    )
                    nc.scalar.dma_start(out_d[k : k + 1, :], os_b[:])

            for _rep in range(repeat):
                scpss.clear()
                attT.clear()
                for w, (b0, wb) in enumerate(WAVE_SPANS):
                    scpss[w] = [
                        sc_psum.tile([wb, 512], F32, tag="scps", name="scps")
                        for _j in range(njs[b0])
                    ]
                    for bwi in range(wb):
                        trace_slot(w, b0, wb, bwi)
                        if bwi == 0 and w > 0:
                            pw = w - 1
                            finish_wave(pw, *WAVE_SPANS[pw])
                finish_wave(len(WAVE_SPANS) - 1, *WAVE_SPANS[-1])

    nc.finalize()
    return nc


_NC_CACHE = {}


def get_nc(caps, widths, repeat: int = 1) -> bass.Bass:
    key = (tuple(caps), tuple(widths), repeat)
    if key not in _NC_CACHE:
        _NC_CACHE[key] = build_nc(caps, widths, repeat=repeat)
    return _NC_CACHE[key]


def plan(lens):
    """Sort batches by length desc; slot k = ranks [8k, 8k+8), one per core.

    Window maxima of the descending sort minimize the summed per-slot caps
    (both the 20*L p1/scores term and the 512*ceil(L/128) p3 term are
    monotone in L, so length-sorting is optimal for the combined cost).
    """
    lens = np.asarray(lens).reshape(B).astype(np.int64)
    order = np.argsort(-lens, kind="stable")
    assign = order.reshape(BPC, NCORES)          # [slot, core] -> batch
    widths = [int(lens[assign[k, 0]]) for k in range(BPC)]
    caps = [(w + 127) // 128 for w in widths]
    return caps, widths, assign


def make_in_maps(nn_outs, batch_lens, context, proj_w, proj_b, caps, widths,
                 assign):
    x_bf = np.asarray(nn_outs, dtype=np.float32).astype(ml_dtypes.bfloat16)
    lens = np.asarray(batch_lens).reshape(B).astype(np.int64)
    wt = np.ascontiguousarray(np.asarray(proj_w, np.float32).T)  # [H, A]
    # wt_sb[p, c*A + a] = wt[128c + p, a]
    wt_host = np.ascontiguousarray(
        wt.reshape(HC, 128, A).transpose(1, 0, 2).reshape(128, HC * A)
    ).astype(ml_dtypes.bfloat16)
    ctx_c = np.asarray(context, np.float32).reshape(AC, 128)
    ctx_host = np.zeros((128, AC, WB, WB), np.float32)
    for a in range(AC):
        for bw in range(WB):
            ctx_host[:, a, bw, bw] = ctx_c[a]
    ctx_host = np.ascontiguousarray(
        ctx_host.reshape(128, AC * WB * WB)
    ).astype(ml_dtypes.bfloat16)
    pb_host = np.ascontiguousarray(
        np.asarray(proj_b, np.float32).reshape(AC, 128).T
    )
    iota = np.arange(S)[None, :]

    xt_w = sum(4 * w for w in widths)
    nat_w = sum(c * 512 for c in caps)
    in_maps = []
    for c in range(NCORES):
        xt_all = np.empty((128, xt_w), ml_dtypes.bfloat16)
        nat_all = np.empty((128, nat_w), ml_dtypes.bfloat16)
        mask = np.zeros((WB, len(WAVE_SPANS) * S), ml_dtypes.bfloat16)
        xo = no = 0
        for k in range(BPC):
            b = assign[k, c]
            W = widths[k]
            xa = x_bf[b, :W, :]                          # [W, H]
            xt_all[:, xo : xo + 4 * W] = (
                xa.T.reshape(HC, 128, W).transpose(1, 0, 2).reshape(128, 4 * W)
            )
            xo += 4 * W
            Wc = caps[k] * 128
            nat_all[:, no : no + caps[k] * 512] = (
                x_bf[b, :Wc, :].reshape(caps[k], 128, 512).transpose(1, 0, 2)
                .reshape(128, caps[k] * 512)
            )
            no += caps[k] * 512
        for w, (b0, wb) in enumerate(WAVE_SPANS):
            for bwi in range(wb):
                b = assign[b0 + bwi, c]
                mask[bwi, w * S : (w + 1) * S] = np.where(
                    iota[0] < lens[b], 0.0, -30000.0
                )
        in_maps.append(
            {
                "xt": xt_all,
                "nat": nat_all,
                "wt": wt_host,
                "ctx": ctx_host,
                "pb": pb_host,
                "mask": mask,
            }
        )
    return in_maps


def run(nn_outs, batch_lens, context, proj_w, proj_b, trace=False, repeat=1,
        **trace_kw):
    from concourse.bass_utils import run_bass_kernel_spmd

    caps, widths, assign = plan(batch_lens)
    nc = get_nc(caps, widths, repeat=repeat)
    in_maps = make_in_maps(
        nn_outs, batch_lens, context, proj_w, proj_b, caps, widths, assign
    )
    res = run_bass_kernel_spmd(
        nc, in_maps, list(range(NCORES)), trace=trace, **trace_kw
    )
    out = np.empty((B, H), np.float32)
    for c in range(NCORES):
        out[assign[:, c]] = res.results[c]["out"]
    return out, res


def kernel(nn_outs, batch_lens, context, proj_w, proj_b):
    out, _ = run(nn_outs, batch_lens, context, proj_w, proj_b, trace=False)
    return out


# revision 7
# speedup vs baseline: 1.0286x; 1.0040x over previous
"""Trainium2 Bass kernel for nn_AttenLayer (ragged-sequence attention pooling).

Math (per batch b, with length L_b):
    proj   = tanh(nn_outs @ W^T + b)           # (S, A)
    scores = proj @ context                     # (S,)
    atten  = masked_softmax(scores, L_b)        # (S,), zeros beyond L_b
    out    = atten @ nn_outs                    # (H,)

Ragged slot-capped data-parallel sharding over 8 cores:
  - All 64 batches are sorted by ceil(L/128) descending; rank window
    [8k, 8k+8) forms SLOT k, one batch per core per slot.  Every core
    runs the same instruction stream sized by the slot caps
    caps[k] = max ceil(L/128) over the window, so only ~Sigma caps*128
    tokens are computed/DMAd per core instead of 8*S.
  - Positions in [L_b, cap*128) are computed on real data but killed by
    the additive -30000 mask before softmax (exactly like full-width).
  - Waves of slots [0-3], [4-5], [6], [7] share a [wb, 512]-per-chunk
    scores PSUM via the zero-padded ctx trick; slot order descending
    guarantees the first writer covers the widest region (start=True).
    The two trailing single-slot waves keep the final (unoverlapped)
    softmax+phase-3 tail as small as possible.
  - ex (unnormalized exp) is transposed on PE and used directly as the
    phase-3 lhsT; the 1/sum normalization is applied to the final
    [1, H] output by the ACT copy (scale=rv), saving a [wb,S] pass.
"""

import sys

for _p in ("/opt/trn_rl_repo",):
    if _p not in sys.path:
        sys.path.insert(0, _p)

import numpy as np
import ml_dtypes

import concourse.bass as bass
from concourse import bacc
import concourse.mybir as mybir
import concourse.tile as tile
from concourse.masks import make_identity

B, S, H, A = 64, 2048, 512, 512
NCORES = 8
BPC = B // NCORES          # slots per core
WAVE_SPANS = [(0, 4), (4, 2), (6, 1), (7, 1)]
WB = 4                     # max wave size (ctx zero-pad layout width)

AC = A // 128              # 4 a-chunks
HC = H // 128              # 4 h-chunks

F32 = mybir.dt.float32
BF16 = mybir.dt.bfloat16


def build_nc(caps, widths, repeat: int = 1) -> bass.Bass:
    caps = [int(c) for c in caps]
    widths = [int(w) for w in widths]
    assert len(caps) == BPC and all(1 <= c <= S // 128 for c in caps)
    assert all(caps[i] >= caps[i + 1] for i in range(BPC - 1))
    assert all(widths[i] >= widths[i + 1] for i in range(BPC - 1))
    assert all((c - 1) * 128 < w <= c * 128 for c, w in zip(caps, widths))
    Wk = widths
    njs = [(w + 511) // 512 for w in Wk]
    xt_off = [0]
    nat_off = [0]
    for k in range(BPC):
        xt_off.append(xt_off[-1] + 4 * Wk[k])
        nat_off.append(nat_off[-1] + caps[k] * 512)

    nc = bacc.Bacc()

    xt_d = nc.declare_dram_parameter("xt", [128, xt_off[-1]], BF16, isOutput=False)
    nat_d = nc.declare_dram_parameter("nat", [128, nat_off[-1]], BF16, isOutput=False)
    # W^T pre-chunked on host: wt[p, c*A + a] = proj_w[a, 128c + p]
    wt_d = nc.declare_dram_parameter("wt", [128, HC * A], BF16, isOutput=False)
    ctx_d = nc.declare_dram_parameter("ctx", [128, AC * WB * WB], BF16, isOutput=False)
    pb_d = nc.declare_dram_parameter("pb", [128, AC], F32, isOutput=False)
    # mask rows regrouped per wave so each wave's rows start at partition 0
    mask_d = nc.declare_dram_parameter(
        "mask", [WB, len(WAVE_SPANS) * S], BF16, isOutput=False
    )
    out_d = nc.declare_dram_parameter("out", [BPC, H], F32, isOutput=True)

    with tile.TileContext(nc) as tc:
        with (
            tc.tile_pool(name="const", bufs=1) as const_pool,
            tc.tile_pool(name="xt", bufs=3) as xt_pool,
            tc.tile_pool(name="nat", bufs=5) as nat_pool,
            tc.tile_pool(name="projT", bufs=10) as proj_pool,
            tc.tile_pool(name="smx", bufs=2) as smx_pool,
            tc.tile_pool(name="attT", bufs=8) as attT_pool,
            tc.tile_pool(name="osb", bufs=4) as os_pool,
            tc.tile_pool(name="p1ps", bufs=2, space="PSUM") as p1_psum,
            tc.tile_pool(name="scps", bufs=4, space="PSUM") as sc_psum,
            tc.tile_pool(name="atps", bufs=1, space="PSUM") as at_psum,
            tc.tile_pool(name="ops", bufs=1, space="PSUM") as out_psum,
        ):
            # ---- constants (DMAs deferred to first use in slot 0) ----
            wt_sb = const_pool.tile([128, HC * A], BF16, tag="wt")
            ctx_sb = const_pool.tile([128, AC * WB * WB], BF16, tag="ctx")
            pb_sb = const_pool.tile([128, AC], F32, tag="pb")
            ident = const_pool.tile([WB, WB], BF16, tag="ident")
            make_identity(nc, ident[:])
            # natural tiles: fixed-size ring; at most 5 are live at once
            # (wave-0's four slots + the first slot of wave 1)
            nat_sb = {}

            mask_sb = const_pool.tile([WB, len(WAVE_SPANS) * S], BF16, tag="mask")
            scpss = {}  # w -> list of scores psum tiles [wb, 512]
            attT = {}   # (w, g) -> [128, 4*WB] bf16, col = wb*jj + bwi
            # last slot-in-wave that writes scores chunk j
            last_writer = {}
            for w, (b0, wb) in enumerate(WAVE_SPANS):
                last_writer[w] = [
                    max(bwi for bwi in range(wb) if njs[b0 + bwi] > j)
                    for j in range(njs[b0])
                ]

            def trace_slot(w, b0, wb, bwi):
                k = b0 + bwi
                W = Wk[k]
                nj = njs[k]
                xt = xt_pool.tile([128, 4, 2048], BF16, tag="xt")
                src = xt_d[:, xt_off[k] : xt_off[k] + 4 * W].rearrange(
                    "p (hc w) -> p hc w", hc=HC
                )
                if k == 0:
                    # interleave wt with the first xt chunks on the SP ring;
                    # the remaining xt pieces stream on the idle Pool ring so
                    # chunk j+1 lands while chunk j computes
                    c1 = min(W, 512)
                    for hc in range(HC):
                        nc.sync.dma_start(
                            wt_sb[:, hc * A : (hc + 1) * A],
                            wt_d[:, hc * A : (hc + 1) * A],
                        )
                        nc.sync.dma_start(xt[:, hc, :c1], src[:, hc, :c1])
                    nc.gpsimd.dma_start(pb_sb[:], pb_d[:])
                    nc.gpsimd.dma_start(ctx_sb[:], ctx_d[:])
                    for j in range(1, nj):
                        c0, c1 = j * 512, min(W, (j + 1) * 512)
                        eng = nc.gpsimd if j % 2 else nc.sync
                        eng.dma_start(xt[:, :, c0:c1], src[:, :, c0:c1])
                    # mask is not needed until the first softmax; keep it
                    # behind the latency-critical xt pieces
                    nc.gpsimd.dma_start(mask_sb[:], mask_d[:])
                else:
                    nc.sync.dma_start(xt[:, :, :W], src[:])
                natk = nat_pool.tile([128, 16 * 512], BF16, tag="nat")
                nat_sb[k] = natk
                nc.gpsimd.dma_start(
                    natk[:, : caps[k] * 512],
                    nat_d[:, nat_off[k] : nat_off[k + 1]],
                )
                pq = []  # software-pipeline: scores MMs trail p1 by 8 a-steps
                for j in range(nj):
                    wj = min(512, W - j * 512)
                    for a in range(AC):
                        ps = p1_psum.tile([128, 512], F32, tag="p1")
                        for hc in range(HC):
                            nc.tensor.matmul(
                                ps[:, :wj],
                                wt_sb[:, hc * A + a * 128 : hc * A + (a + 1) * 128],
                                xt[:, hc, j * 512 : j * 512 + wj],
                                start=(hc == 0),
                                stop=(hc == HC - 1),
                            )
                        if len(pq) >= 8:
                            pq.pop(0)()
                        pt = proj_pool.tile([128, 512], BF16, tag="projT")
                        nc.scalar.activation(
                            pt[:, :wj],
                            ps[:, :wj],
                            mybir.ActivationFunctionType.Tanh,
                            bias=pb_sb[:, a : a + 1],
                        )
                        # ctx col bwi is context's a-chunk, others zero, so only
                        # row bwi of the wave's scores psum accumulates slot k.
                        def pend(pt=pt, j=j, a=a, wj=wj):
                            nc.tensor.matmul(
                                scpss[w][j][:, :wj],
                                ctx_sb[:, (a * WB + bwi) * WB : (a * WB + bwi) * WB + wb],
                                pt[:, :wj],
                                start=(bwi == 0 and a == 0),
                                stop=(bwi == last_writer[w][j] and a == AC - 1),
                            )
                        pq.append(pend)
                for f in pq:
                    f()

            def finish_wave(w, b0, wb):
                Wmax = Wk[b0]
                njw = njs[b0]
                capm = caps[b0]
                mw = mask_sb[:wb, w * S : (w + 1) * S]
                # masked scores: scm = scores + additive mask, chunkwise
                scm = smx_pool.tile([wb, S], F32, tag="scm")
                pmax = smx_pool.tile([wb, 4], F32, tag="pmax")
                for j in range(njw):
                    wjj = min(512, Wmax - j * 512)
                    sl = slice(j * 512, j * 512 + wjj)
                    nc.vector.tensor_tensor(
                        out=scm[:, sl], in0=scpss[w][j][:, :wjj],
                        in1=mw[:, sl], op=mybir.AluOpType.add,
                    )
                    nc.vector.reduce_max(
                        pmax[:, j : j + 1], scm[:, sl], axis=mybir.AxisListType.X
                    )
                mx = smx_pool.tile([wb, 1], F32, tag="mx")
                nc.vector.reduce_max(
                    mx[:], pmax[:, :njw], axis=mybir.AxisListType.X, negate=True
                )
                ex = smx_pool.tile([wb, S], BF16, tag="ex")
                rs = smx_pool.tile([wb, 1], F32, tag="rs")
                nc.scalar.activation(
                    ex[:, :Wmax],
                    scm[:, :Wmax],
                    mybir.ActivationFunctionType.Exp,
                    bias=mx[:],
                    accum_out=rs[:],
                )
                rv = smx_pool.tile([wb, 1], F32, tag="rv")
                nc.vector.reciprocal(rv[:], rs[:])
                # move rv to partition 0 (tiny SBUF->SBUF DMA) so the final
                # output copy can read it as a per-partition scale
                rvT = smx_pool.tile([1, WB], F32, tag="rvT")
                nc.gpsimd.dma_start(rvT[0:1, :wb], rv[:, 0:1])
                # zero the ex tail beyond the wave width so the last
                # 128-chunk transposes clean zeros
                if capm * 128 > Wmax:
                    nc.vector.memset(ex[:, Wmax : capm * 128], 0.0)
                # transpose ex chunks [wb, 128] -> [128, wb] on PE (bf16);
                # column stride padded to 2 so bf16 PSUM offsets stay 4B-aligned
                cs = wb if wb % 2 == 0 else wb + 1
                for g in range((capm + 3) // 4):
                    na = min(4, capm - 4 * g)
                    aps = at_psum.tile([128, 4 * WB], BF16, tag="atps")
                    for jj in range(na):
                        ch = 4 * g + jj
                        nc.tensor.transpose(
                            aps[:, jj * cs : jj * cs + wb],
                            ex[:, ch * 128 : (ch + 1) * 128],
                            ident[:wb, :wb],
                        )
                    att_sb = attT_pool.tile([128, 4 * WB], BF16, tag="attT")
                    nc.vector.tensor_copy(
                        att_sb[:, : na * cs].rearrange(
                            "p (n c) -> p n c", c=cs
                        )[:, :, :wb],
                        aps[:, : na * cs].rearrange(
                            "p (n c) -> p n c", c=cs
                        )[:, :, :wb],
                    )
                    attT[(w, g)] = att_sb
                # phase 3: out[k] = (sum_s ex[s] * x[s, :]) * rv
                # smallest slot first so its output DMA lands earliest
                for bwi in reversed(range(wb)):
                    k = b0 + bwi
                    # single-row waves borrow a scores-ring bank (free by
                    # now) so the tail p3 never waits on the out-psum recycle
                    pool = sc_psum if wb == 1 else out_psum
                    tag = "scps" if wb == 1 else "ops"
                    ops = pool.tile([1, H], F32, tag=tag, name="ops")
                    for n in range(caps[k]):
                        col = (n % 4) * cs + bwi
                        nc.tensor.matmul(
                            ops[:],
                            attT[(w, n // 4)][:, col : col + 1],
                            nat_sb[k][:, n * 512 : (n + 1) * 512],
                            start=(n == 0),
                            stop=(n == caps[k] - 1),
                        )
                    os_b = os_pool.tile([1, H], F32, tag="os")
                    nc.vector.tensor_scalar_mul(
                        os_b[:], ops[:], rvT[0:1, bwi : bwi + 1]
                    )
                    nc.scalar.dma_start(out_d[k : k + 1, :], os_b[:])

            for _rep in range(repeat):
                scpss.clear()
                attT.clear()
                for w, (b0, wb) in enumerate(WAVE_SPANS):
                    scpss[w] = [
                        sc_psum.tile([wb, 512], F32, tag="scps", name="scps")
                        for _j in range(njs[b0])
                    ]
                    for bwi in range(wb):
                        trace_slot(w, b0, wb, bwi)
                        if bwi == 0 and w > 0:
                            pw = w - 1
                            finish_wave(pw, *WAVE_SPANS[pw])
                finish_wave(len(WAVE_SPANS) - 1, *WAVE_SPANS[-1])

    nc.finalize()
    return nc


_NC_CACHE = {}


def get_nc(caps, widths, repeat: int = 1) -> bass.Bass:
    key = (tuple(caps), tuple(widths), repeat)
    if key not in _NC_CACHE:
        _NC_CACHE[key] = build_nc(caps, widths, repeat=repeat)
    return _NC_CACHE[key]


def plan(lens):
    """Sort batches by length desc; slot k = ranks [8k, 8k+8), one per core.

    Window maxima of the descending sort minimize the summed per-slot caps
    (both the 20*L p1/scores term and the 512*ceil(L/128) p3 term are
    monotone in L, so length-sorting is optimal for the combined cost).
    """
    lens = np.asarray(lens).reshape(B).astype(np.int64)
    order = np.argsort(-lens, kind="stable")
    assign = order.reshape(BPC, NCORES)          # [slot, core] -> batch
    widths = [int(lens[assign[k, 0]]) for k in range(BPC)]
    caps = [(w + 127) // 128 for w in widths]
    return caps, widths, assign


def make_in_maps(nn_outs, batch_lens, context, proj_w, proj_b, caps, widths,
                 assign):
    x_bf = np.asarray(nn_outs, dtype=np.float32).astype(ml_dtypes.bfloat16)
    lens = np.asarray(batch_lens).reshape(B).astype(np.int64)
    wt = np.ascontiguousarray(np.asarray(proj_w, np.float32).T)  # [H, A]
    # wt_sb[p, c*A + a] = wt[128c + p, a]
    wt_host = np.ascontiguousarray(
        wt.reshape(HC, 128, A).transpose(1, 0, 2).reshape(128, HC * A)
    ).astype(ml_dtypes.bfloat16)
    ctx_c = np.asarray(context, np.float32).reshape(AC, 128)
    ctx_host = np.zeros((128, AC, WB, WB), np.float32)
    for a in range(AC):
        for bw in range(WB):
            ctx_host[:, a, bw, bw] = ctx_c[a]
    ctx_host = np.ascontiguousarray(
        ctx_host.reshape(128, AC * WB * WB)
    ).astype(ml_dtypes.bfloat16)
    pb_host = np.ascontiguousarray(
        np.asarray(proj_b, np.float32).reshape(AC, 128).T
    )
    iota = np.arange(S)[None, :]

    xt_w = sum(4 * w for w in widths)
    nat_w = sum(c * 512 for c in caps)
    in_maps = []
    for c in range(NCORES):
        xt_all = np.empty((128, xt_w), ml_dtypes.bfloat16)
        nat_all = np.empty((128, nat_w), ml_dtypes.bfloat16)
        mask = np.zeros((WB, len(WAVE_SPANS) * S), ml_dtypes.bfloat16)
        xo = no = 0
        for k in range(BPC):
            b = assign[k, c]
            W = widths[k]
            xa = x_bf[b, :W, :]                          # [W, H]
            xt_all[:, xo : xo + 4 * W] = (
                xa.T.reshape(HC, 128, W).transpose(1, 0, 2).reshape(128, 4 * W)
            )
            xo += 4 * W
            Wc = caps[k] * 128
            nat_all[:, no : no + caps[k] * 512] = (
                x_bf[b, :Wc, :].reshape(caps[k], 128, 512).transpose(1, 0, 2)
                .reshape(128, caps[k] * 512)
            )
            no += caps[k] * 512
        for w, (b0, wb) in enumerate(WAVE_SPANS):
            for bwi in range(wb):
                b = assign[b0 + bwi, c]
                mask[bwi, w * S : (w + 1) * S] = np.where(
                    iota[0] < lens[b], 0.0, -30000.0
                )
        in_maps.append(
            {
                "xt": xt_all,
                "nat": nat_all,
                "wt": wt_host,
                "ctx": ctx_host,
                "pb": pb_host,
                "mask": mask,
            }
        )
    return in_maps


def run(nn_outs, batch_lens, context, proj_w, proj_b, trace=False, repeat=1,
        **trace_kw):
    from concourse.bass_utils import run_bass_kernel_spmd

    caps, widths, assign = plan(batch_lens)
    nc = get_nc(caps, widths, repeat=repeat)
    in_maps = make_in_maps(
        nn_outs, batch_lens, context, proj_w, proj_b, caps, widths, assign
    )
    res = run_bass_kernel_spmd(
        nc, in_maps, list(range(NCORES)), trace=trace, **trace_kw
    )
    out = np.empty((B, H), np.float32)
    for c in range(NCORES):
        out[assign[:, c]] = res.results[c]["out"]
    return out, res


def kernel(nn_outs, batch_lens, context, proj_w, proj_b):
    out, _ = run(nn_outs, batch_lens, context, proj_w, proj_b, trace=False)
    return out


# revision 8
# speedup vs baseline: 1.0337x; 1.0049x over previous
"""Trainium2 Bass kernel for nn_AttenLayer (ragged-sequence attention pooling).

Math (per batch b, with length L_b):
    proj   = tanh(nn_outs @ W^T + b)           # (S, A)
    scores = proj @ context                     # (S,)
    atten  = masked_softmax(scores, L_b)        # (S,), zeros beyond L_b
    out    = atten @ nn_outs                    # (H,)

Ragged slot-capped data-parallel sharding over 8 cores:
  - All 64 batches are sorted by ceil(L/128) descending; rank window
    [8k, 8k+8) forms SLOT k, one batch per core per slot.  Every core
    runs the same instruction stream sized by the slot caps
    caps[k] = max ceil(L/128) over the window, so only ~Sigma caps*128
    tokens are computed/DMAd per core instead of 8*S.
  - Positions in [L_b, cap*128) are computed on real data but killed by
    the additive -30000 mask before softmax (exactly like full-width).
  - Waves of slots [0-3], [4-5], [6], [7] share a [wb, 512]-per-chunk
    scores PSUM via the zero-padded ctx trick; slot order descending
    guarantees the first writer covers the widest region (start=True).
    The two trailing single-slot waves keep the final (unoverlapped)
    softmax+phase-3 tail as small as possible.
  - ex (unnormalized exp) is transposed on PE and used directly as the
    phase-3 lhsT; the 1/sum normalization is applied to the final
    [1, H] output by the ACT copy (scale=rv), saving a [wb,S] pass.
"""

import sys

for _p in ("/opt/trn_rl_repo",):
    if _p not in sys.path:
        sys.path.insert(0, _p)

import numpy as np
import ml_dtypes

import concourse.bass as bass
from concourse import bacc
import concourse.mybir as mybir
import concourse.tile as tile
from concourse.masks import make_identity

B, S, H, A = 64, 2048, 512, 512
NCORES = 8
BPC = B // NCORES          # slots per core
WAVE_SPANS = [(0, 4), (4, 2), (6, 1), (7, 1)]
WB = 4                     # max wave size (ctx zero-pad layout width)

AC = A // 128              # 4 a-chunks
HC = H // 128              # 4 h-chunks

F32 = mybir.dt.float32
BF16 = mybir.dt.bfloat16


def build_nc(caps, widths, repeat: int = 1) -> bass.Bass:
    caps = [int(c) for c in caps]
    widths = [int(w) for w in widths]
    assert len(caps) == BPC and all(1 <= c <= S // 128 for c in caps)
    assert all(caps[i] >= caps[i + 1] for i in range(BPC - 1))
    assert all(widths[i] >= widths[i + 1] for i in range(BPC - 1))
    assert all((c - 1) * 128 < w <= c * 128 for c, w in zip(caps, widths))
    Wk = widths
    njs = [(w + 511) // 512 for w in Wk]
    xt_off = [0]
    nat_off = [0]
    for k in range(BPC):
        xt_off.append(xt_off[-1] + 4 * Wk[k])
        nat_off.append(nat_off[-1] + caps[k] * 512)

    nc = bacc.Bacc()

    xt_d = nc.declare_dram_parameter("xt", [128, xt_off[-1]], BF16, isOutput=False)
    nat_d = nc.declare_dram_parameter("nat", [128, nat_off[-1]], BF16, isOutput=False)
    # W^T pre-chunked on host: wt[p, c*A + a] = proj_w[a, 128c + p]
    wt_d = nc.declare_dram_parameter("wt", [128, HC * A], BF16, isOutput=False)
    ctx_d = nc.declare_dram_parameter("ctx", [128, AC * WB * WB], BF16, isOutput=False)
    pb_d = nc.declare_dram_parameter("pb", [128, AC], F32, isOutput=False)
    # mask rows regrouped per wave so each wave's rows start at partition 0
    mask_d = nc.declare_dram_parameter(
        "mask", [WB, len(WAVE_SPANS) * S], BF16, isOutput=False
    )
    out_d = nc.declare_dram_parameter("out", [BPC, H], F32, isOutput=True)

    with tile.TileContext(nc) as tc:
        with (
            tc.tile_pool(name="const", bufs=1) as const_pool,
            tc.tile_pool(name="xt", bufs=3) as xt_pool,
            tc.tile_pool(name="nat", bufs=5) as nat_pool,
            tc.tile_pool(name="projT", bufs=10) as proj_pool,
            tc.tile_pool(name="smx", bufs=2) as smx_pool,
            tc.tile_pool(name="attT", bufs=8) as attT_pool,
            tc.tile_pool(name="osb", bufs=4) as os_pool,
            tc.tile_pool(name="p1ps", bufs=2, space="PSUM") as p1_psum,
            tc.tile_pool(name="scps", bufs=4, space="PSUM") as sc_psum,
            tc.tile_pool(name="atps", bufs=1, space="PSUM") as at_psum,
            tc.tile_pool(name="ops", bufs=1, space="PSUM") as out_psum,
        ):
            # ---- constants (DMAs deferred to first use in slot 0) ----
            wt_sb = const_pool.tile([128, HC * A], BF16, tag="wt")
            ctx_sb = const_pool.tile([128, AC * WB * WB], BF16, tag="ctx")
            pb_sb = const_pool.tile([128, AC], F32, tag="pb")
            ident = const_pool.tile([WB, WB], BF16, tag="ident")
            make_identity(nc, ident[:])
            # natural tiles: fixed-size ring; at most 5 are live at once
            # (wave-0's four slots + the first slot of wave 1)
            nat_sb = {}

            mask_sb = const_pool.tile([WB, len(WAVE_SPANS) * S], BF16, tag="mask")
            scpss = {}  # w -> list of scores psum tiles [wb, 512]
            attT = {}   # (w, g) -> [128, 4*WB] bf16, col = wb*jj + bwi
            # last slot-in-wave that writes scores chunk j
            last_writer = {}
            for w, (b0, wb) in enumerate(WAVE_SPANS):
                last_writer[w] = [
                    max(bwi for bwi in range(wb) if njs[b0 + bwi] > j)
                    for j in range(njs[b0])
                ]

            def trace_slot(w, b0, wb, bwi):
                k = b0 + bwi
                W = Wk[k]
                nj = njs[k]
                xt = xt_pool.tile([128, 4, 2048], BF16, tag="xt")
                src = xt_d[:, xt_off[k] : xt_off[k] + 4 * W].rearrange(
                    "p (hc w) -> p hc w", hc=HC
                )
                if k == 0:
                    # interleave wt with the first xt chunks on the SP ring;
                    # the remaining xt pieces stream on the idle Pool ring so
                    # chunk j+1 lands while chunk j computes
                    c1 = min(W, 512)
                    for hc in range(HC):
                        nc.sync.dma_start(
                            wt_sb[:, hc * A : (hc + 1) * A],
                            wt_d[:, hc * A : (hc + 1) * A],
                        )
                        nc.sync.dma_start(xt[:, hc, :c1], src[:, hc, :c1])
                    nc.gpsimd.dma_start(pb_sb[:], pb_d[:])
                    nc.gpsimd.dma_start(ctx_sb[:], ctx_d[:])
                    for j in range(1, nj):
                        c0, c1 = j * 512, min(W, (j + 1) * 512)
                        eng = nc.gpsimd if j % 2 else nc.sync
                        eng.dma_start(xt[:, :, c0:c1], src[:, :, c0:c1])
                    # mask is not needed until the first softmax; keep it
                    # behind the latency-critical xt pieces
                    nc.gpsimd.dma_start(mask_sb[:], mask_d[:])
                else:
                    nc.sync.dma_start(xt[:, :, :W], src[:])
                natk = nat_pool.tile([128, 16 * 512], BF16, tag="nat")
                nat_sb[k] = natk
                nc.gpsimd.dma_start(
                    natk[:, : caps[k] * 512],
                    nat_d[:, nat_off[k] : nat_off[k + 1]],
                )
                pq = []  # software-pipeline: scores MMs trail p1 by 8 a-steps
                for j in range(nj):
                    wj = min(512, W - j * 512)
                    for a in range(AC):
                        ps = p1_psum.tile([128, 512], F32, tag="p1")
                        for hc in range(HC):
                            nc.tensor.matmul(
                                ps[:, :wj],
                                wt_sb[:, hc * A + a * 128 : hc * A + (a + 1) * 128],
                                xt[:, hc, j * 512 : j * 512 + wj],
                                start=(hc == 0),
                                stop=(hc == HC - 1),
                            )
                        if len(pq) >= 8:
                            pq.pop(0)()
                        pt = proj_pool.tile([128, 512], BF16, tag="projT")
                        nc.scalar.activation(
                            pt[:, :wj],
                            ps[:, :wj],
                            mybir.ActivationFunctionType.Tanh,
                            bias=pb_sb[:, a : a + 1],
                        )
                        # ctx col bwi is context's a-chunk, others zero, so only
                        # row bwi of the wave's scores psum accumulates slot k.
                        def pend(pt=pt, j=j, a=a, wj=wj):
                            nc.tensor.matmul(
                                scpss[w][j][:, :wj],
                                ctx_sb[:, (a * WB + bwi) * WB : (a * WB + bwi) * WB + wb],
                                pt[:, :wj],
                                start=(bwi == 0 and a == 0),
                                stop=(bwi == last_writer[w][j] and a == AC - 1),
                            )
                        pq.append(pend)
                for f in pq:
                    f()

            def finish_wave(w, b0, wb):
                Wmax = Wk[b0]
                njw = njs[b0]
                capm = caps[b0]
                mw = mask_sb[:wb, w * S : (w + 1) * S]
                # masked scores: scm = scores + additive mask, chunkwise
                scm = smx_pool.tile([wb, S], F32, tag="scm")
                pmax = smx_pool.tile([wb, 4], F32, tag="pmax")
                for j in range(njw):
                    wjj = min(512, Wmax - j * 512)
                    sl = slice(j * 512, j * 512 + wjj)
                    nc.vector.tensor_tensor(
                        out=scm[:, sl], in0=scpss[w][j][:, :wjj],
                        in1=mw[:, sl], op=mybir.AluOpType.add,
                    )
                    nc.vector.reduce_max(
                        pmax[:, j : j + 1], scm[:, sl], axis=mybir.AxisListType.X
                    )
                mx = smx_pool.tile([wb, 1], F32, tag="mx")
                nc.vector.reduce_max(
                    mx[:], pmax[:, :njw], axis=mybir.AxisListType.X, negate=True
                )
                ex = smx_pool.tile([wb, S], BF16, tag="ex")
                rs = smx_pool.tile([wb, 1], F32, tag="rs")
                nc.scalar.activation(
                    ex[:, :Wmax],
                    scm[:, :Wmax],
                    mybir.ActivationFunctionType.Exp,
                    bias=mx[:],
                    accum_out=rs[:],
                )
                rv = smx_pool.tile([wb, 1], F32, tag="rv")
                nc.vector.reciprocal(rv[:], rs[:])
                if wb > 1:
                    # move rv to partition 0 (tiny SBUF->SBUF DMA) so the
                    # final output copy can read it as a per-partition scale
                    rvT = smx_pool.tile([1, WB], F32, tag="rvT")
                    nc.gpsimd.dma_start(rvT[0:1, :wb], rv[:, 0:1])
                else:
                    rvT = rv  # single-row wave: already at partition 0
                # zero the ex tail beyond the wave width so the last
                # 128-chunk transposes clean zeros
                if capm * 128 > Wmax:
                    nc.vector.memset(ex[:, Wmax : capm * 128], 0.0)
                # transpose ex chunks [wb, 128] -> [128, wb] on PE (bf16);
                # column stride padded to 2 so bf16 PSUM offsets stay 4B-aligned
                cs = wb if wb % 2 == 0 else wb + 1
                for g in range((capm + 3) // 4):
                    na = min(4, capm - 4 * g)
                    aps = at_psum.tile([128, 4 * WB], BF16, tag="atps")
                    for jj in range(na):
                        ch = 4 * g + jj
                        nc.tensor.transpose(
                            aps[:, jj * cs : jj * cs + wb],
                            ex[:, ch * 128 : (ch + 1) * 128],
                            ident[:wb, :wb],
                        )
                    att_sb = attT_pool.tile([128, 4 * WB], BF16, tag="attT")
                    nc.vector.tensor_copy(
                        att_sb[:, : na * cs].rearrange(
                            "p (n c) -> p n c", c=cs
                        )[:, :, :wb],
                        aps[:, : na * cs].rearrange(
                            "p (n c) -> p n c", c=cs
                        )[:, :, :wb],
                    )
                    attT[(w, g)] = att_sb
                # phase 3: out[k] = (sum_s ex[s] * x[s, :]) * rv
                # smallest slot first so its output DMA lands earliest
                for bwi in reversed(range(wb)):
                    k = b0 + bwi
                    # single-row waves borrow a scores-ring bank (free by
                    # now) so the tail p3 never waits on the out-psum recycle
                    pool = sc_psum if wb == 1 else out_psum
                    tag = "scps" if wb == 1 else "ops"
                    ops = pool.tile([1, H], F32, tag=tag, name="ops")
                    for n in range(caps[k]):
                        col = (n % 4) * cs + bwi
                        nc.tensor.matmul(
                            ops[:],
                            attT[(w, n // 4)][:, col : col + 1],
                            nat_sb[k][:, n * 512 : (n + 1) * 512],
                            start=(n == 0),
                            stop=(n == caps[k] - 1),
                        )
                    os_b = os_pool.tile([1, H], F32, tag="os")
                    if wb == 1 and w % 2 == 0:
                        # alternate engines so consecutive single-slot waves'
                        # output scales run in parallel at the kernel tail
                        nc.scalar.activation(
                            os_b[:], ops[:],
                            mybir.ActivationFunctionType.Copy,
                            scale=rvT[0:1, 0:1],
                        )
                    else:
                        nc.vector.tensor_scalar_mul(
                            os_b[:], ops[:], rvT[0:1, bwi : bwi + 1]
                        )
                    nc.scalar.dma_start(out_d[k : k + 1, :], os_b[:])

            for _rep in range(repeat):
                scpss.clear()
                attT.clear()
                fired = 0
                for w, (b0, wb) in enumerate(WAVE_SPANS):
                    scpss[w] = [
                        sc_psum.tile([wb, 512], F32, tag="scps", name="scps")
                        for _j in range(njs[b0])
                    ]
                    for bwi in range(wb):
                        trace_slot(w, b0, wb, bwi)
                        # fire pending finish_waves, but keep the last two
                        # tiny slots' p1 ahead of wave-1's finish so the
                        # final softmax chains start as early as possible
                        if bwi == 0 and w > 0 and w < 2:
                            finish_wave(fired, *WAVE_SPANS[fired])
                            fired += 1
                while fired < len(WAVE_SPANS):
                    finish_wave(fired, *WAVE_SPANS[fired])
                    fired += 1

    nc.finalize()
    return nc


_NC_CACHE = {}


def get_nc(caps, widths, repeat: int = 1) -> bass.Bass:
    key = (tuple(caps), tuple(widths), repeat)
    if key not in _NC_CACHE:
        _NC_CACHE[key] = build_nc(caps, widths, repeat=repeat)
    return _NC_CACHE[key]


def plan(lens):
    """Sort batches by length desc; slot k = ranks [8k, 8k+8), one per core.

    Window maxima of the descending sort minimize the summed per-slot caps
    (both the 20*L p1/scores term and the 512*ceil(L/128) p3 term are
    monotone in L, so length-sorting is optimal for the combined cost).
    """
    lens = np.asarray(lens).reshape(B).astype(np.int64)
    order = np.argsort(-lens, kind="stable")
    assign = order.reshape(BPC, NCORES)          # [slot, core] -> batch
    widths = [int(lens[assign[k, 0]]) for k in range(BPC)]
    caps = [(w + 127) // 128 for w in widths]
    return caps, widths, assign


def make_in_maps(nn_outs, batch_lens, context, proj_w, proj_b, caps, widths,
                 assign):
    x_bf = np.asarray(nn_outs, dtype=np.float32).astype(ml_dtypes.bfloat16)
    lens = np.asarray(batch_lens).reshape(B).astype(np.int64)
    wt = np.ascontiguousarray(np.asarray(proj_w, np.float32).T)  # [H, A]
    # wt_sb[p, c*A + a] = wt[128c + p, a]
    wt_host = np.ascontiguousarray(
        wt.reshape(HC, 128, A).transpose(1, 0, 2).reshape(128, HC * A)
    ).astype(ml_dtypes.bfloat16)
    ctx_c = np.asarray(context, np.float32).reshape(AC, 128)
    ctx_host = np.zeros((128, AC, WB, WB), np.float32)
    for a in range(AC):
        for bw in range(WB):
            ctx_host[:, a, bw, bw] = ctx_c[a]
    ctx_host = np.ascontiguousarray(
        ctx_host.reshape(128, AC * WB * WB)
    ).astype(ml_dtypes.bfloat16)
    pb_host = np.ascontiguousarray(
        np.asarray(proj_b, np.float32).reshape(AC, 128).T
    )
    iota = np.arange(S)[None, :]

    xt_w = sum(4 * w for w in widths)
    nat_w = sum(c * 512 for c in caps)
    in_maps = []
    for c in range(NCORES):
        xt_all = np.empty((128, xt_w), ml_dtypes.bfloat16)
        nat_all = np.empty((128, nat_w), ml_dtypes.bfloat16)
        mask = np.zeros((WB, len(WAVE_SPANS) * S), ml_dtypes.bfloat16)
        xo = no = 0
        for k in range(BPC):
            b = assign[k, c]
            W = widths[k]
            xa = x_bf[b, :W, :]                          # [W, H]
            xt_all[:, xo : xo + 4 * W] = (
                xa.T.reshape(HC, 128, W).transpose(1, 0, 2).reshape(128, 4 * W)
            )
            xo += 4 * W
            Wc = caps[k] * 128
            nat_all[:, no : no + caps[k] * 512] = (
                x_bf[b, :Wc, :].reshape(caps[k], 128, 512).transpose(1, 0, 2)
                .reshape(128, caps[k] * 512)
            )
            no += caps[k] * 512
        for w, (b0, wb) in enumerate(WAVE_SPANS):
            for bwi in range(wb):
                b = assign[b0 + bwi, c]
                mask[bwi, w * S : (w + 1) * S] = np.where(
                    iota[0] < lens[b], 0.0, -30000.0
                )
        in_maps.append(
            {
                "xt": xt_all,
                "nat": nat_all,
                "wt": wt_host,
                "ctx": ctx_host,
                "pb": pb_host,
                "mask": mask,
            }
        )
    return in_maps


def run(nn_outs, batch_lens, context, proj_w, proj_b, trace=False, repeat=1,
        **trace_kw):
    from concourse.bass_utils import run_bass_kernel_spmd

    caps, widths, assign = plan(batch_lens)
    nc = get_nc(caps, widths, repeat=repeat)
    in_maps = make_in_maps(
        nn_outs, batch_lens, context, proj_w, proj_b, caps, widths, assign
    )
    res = run_bass_kernel_spmd(
        nc, in_maps, list(range(NCORES)), trace=trace, **trace_kw
    )
    out = np.empty((B, H), np.float32)
    for c in range(NCORES):
        out[assign[:, c]] = res.results[c]["out"]
    return out, res


def kernel(nn_outs, batch_lens, context, proj_w, proj_b):
    out, _ = run(nn_outs, batch_lens, context, proj_w, proj_b, trace=False)
    return out


# revision 9
# speedup vs baseline: 1.0352x; 1.0014x over previous
"""Trainium2 Bass kernel for nn_AttenLayer (ragged-sequence attention pooling).

Math (per batch b, with length L_b):
    proj   = tanh(nn_outs @ W^T + b)           # (S, A)
    scores = proj @ context                     # (S,)
    atten  = masked_softmax(scores, L_b)        # (S,), zeros beyond L_b
    out    = atten @ nn_outs                    # (H,)

Ragged slot-capped data-parallel sharding over 8 cores:
  - All 64 batches are sorted by ceil(L/128) descending; rank window
    [8k, 8k+8) forms SLOT k, one batch per core per slot.  Every core
    runs the same instruction stream sized by the slot caps
    caps[k] = max ceil(L/128) over the window, so only ~Sigma caps*128
    tokens are computed/DMAd per core instead of 8*S.
  - Positions in [L_b, cap*128) are computed on real data but killed by
    the additive -30000 mask before softmax (exactly like full-width).
  - Waves of slots [0-3], [4-5], [6], [7] share a [wb, 512]-per-chunk
    scores PSUM via the zero-padded ctx trick; slot order descending
    guarantees the first writer covers the widest region (start=True).
    The two trailing single-slot waves keep the final (unoverlapped)
    softmax+phase-3 tail as small as possible.
  - ex (unnormalized exp) is transposed on PE and used directly as the
    phase-3 lhsT; the 1/sum normalization is applied to the final
    [1, H] output by the ACT copy (scale=rv), saving a [wb,S] pass.
"""

import sys

for _p in ("/opt/trn_rl_repo",):
    if _p not in sys.path:
        sys.path.insert(0, _p)

import numpy as np
import ml_dtypes

import concourse.bass as bass
from concourse import bacc
import concourse.mybir as mybir
import concourse.tile as tile
from concourse.masks import make_identity

B, S, H, A = 64, 2048, 512, 512
NCORES = 8
BPC = B // NCORES          # slots per core
WAVE_SPANS = [(0, 4), (4, 2), (6, 1), (7, 1)]
WB = 4                     # max wave size (ctx zero-pad layout width)

AC = A // 128              # 4 a-chunks
HC = H // 128              # 4 h-chunks

F32 = mybir.dt.float32
BF16 = mybir.dt.bfloat16


def build_nc(caps, widths, repeat: int = 1) -> bass.Bass:
    caps = [int(c) for c in caps]
    widths = [int(w) for w in widths]
    assert len(caps) == BPC and all(1 <= c <= S // 128 for c in caps)
    assert all(caps[i] >= caps[i + 1] for i in range(BPC - 1))
    assert all(widths[i] >= widths[i + 1] for i in range(BPC - 1))
    assert all((c - 1) * 128 < w <= c * 128 for c, w in zip(caps, widths))
    Wk = widths
    njs = [(w + 511) // 512 for w in Wk]
    xt_off = [0]
    nat_off = [0]
    for k in range(BPC):
        xt_off.append(xt_off[-1] + 4 * Wk[k])
        nat_off.append(nat_off[-1] + caps[k] * 512)

    nc = bacc.Bacc()

    xt_d = nc.declare_dram_parameter("xt", [128, xt_off[-1]], BF16, isOutput=False)
    nat_d = nc.declare_dram_parameter("nat", [128, nat_off[-1]], BF16, isOutput=False)
    # W^T pre-chunked on host: wt[p, c*A + a] = proj_w[a, 128c + p]
    wt_d = nc.declare_dram_parameter("wt", [128, HC * A], BF16, isOutput=False)
    ctx_d = nc.declare_dram_parameter("ctx", [128, AC * WB * WB], BF16, isOutput=False)
    pb_d = nc.declare_dram_parameter("pb", [128, AC], F32, isOutput=False)
    # mask rows regrouped per wave so each wave's rows start at partition 0
    mask_d = nc.declare_dram_parameter(
        "mask", [WB, len(WAVE_SPANS) * S], BF16, isOutput=False
    )
    out_d = nc.declare_dram_parameter("out", [BPC, H], F32, isOutput=True)

    with tile.TileContext(nc) as tc:
        with (
            tc.tile_pool(name="const", bufs=1) as const_pool,
            tc.tile_pool(name="xt", bufs=3) as xt_pool,
            tc.tile_pool(name="nat", bufs=5) as nat_pool,
            tc.tile_pool(name="projT", bufs=10) as proj_pool,
            tc.tile_pool(name="smx", bufs=2) as smx_pool,
            tc.tile_pool(name="attT", bufs=8) as attT_pool,
            tc.tile_pool(name="osb", bufs=4) as os_pool,
            tc.tile_pool(name="p1ps", bufs=2, space="PSUM") as p1_psum,
            tc.tile_pool(name="scps", bufs=4, space="PSUM") as sc_psum,
            tc.tile_pool(name="atps", bufs=1, space="PSUM") as at_psum,
            tc.tile_pool(name="ops", bufs=1, space="PSUM") as out_psum,
        ):
            # ---- constants (DMAs deferred to first use in slot 0) ----
            wt_sb = const_pool.tile([128, HC * A], BF16, tag="wt")
            ctx_sb = const_pool.tile([128, AC * WB * WB], BF16, tag="ctx")
            pb_sb = const_pool.tile([128, AC], F32, tag="pb")
            ident = const_pool.tile([WB, WB], BF16, tag="ident")
            make_identity(nc, ident[:])
            # natural tiles: fixed-size ring; at most 5 are live at once
            # (wave-0's four slots + the first slot of wave 1)
            nat_sb = {}

            mask_sb = const_pool.tile([WB, len(WAVE_SPANS) * S], BF16, tag="mask")
            scpss = {}  # w -> list of scores psum tiles [wb, 512]
            attT = {}   # (w, g) -> [128, 4*WB] bf16, col = wb*jj + bwi
            # last slot-in-wave that writes scores chunk j
            last_writer = {}
            for w, (b0, wb) in enumerate(WAVE_SPANS):
                last_writer[w] = [
                    max(bwi for bwi in range(wb) if njs[b0 + bwi] > j)
                    for j in range(njs[b0])
                ]

            def trace_slot(w, b0, wb, bwi):
                k = b0 + bwi
                W = Wk[k]
                nj = njs[k]
                xt = xt_pool.tile([128, 4, 2048], BF16, tag="xt")
                src = xt_d[:, xt_off[k] : xt_off[k] + 4 * W].rearrange(
                    "p (hc w) -> p hc w", hc=HC
                )
                if k == 0:
                    # interleave wt with the first xt chunks on the SP ring;
                    # the remaining xt pieces stream on the idle Pool ring so
                    # chunk j+1 lands while chunk j computes
                    c1 = min(W, 512)
                    for hc in range(HC):
                        nc.sync.dma_start(
                            wt_sb[:, hc * A : (hc + 1) * A],
                            wt_d[:, hc * A : (hc + 1) * A],
                        )
                        nc.sync.dma_start(xt[:, hc, :c1], src[:, hc, :c1])
                    nc.gpsimd.dma_start(pb_sb[:], pb_d[:])
                    nc.gpsimd.dma_start(ctx_sb[:], ctx_d[:])
                    for j in range(1, nj):
                        c0, c1 = j * 512, min(W, (j + 1) * 512)
                        eng = nc.gpsimd if j % 2 else nc.sync
                        eng.dma_start(xt[:, :, c0:c1], src[:, :, c0:c1])
                    # mask is not needed until the first softmax; keep it
                    # behind the latency-critical xt pieces
                    nc.gpsimd.dma_start(mask_sb[:], mask_d[:])
                else:
                    nc.sync.dma_start(xt[:, :, :W], src[:])
                natk = nat_pool.tile([128, 16 * 512], BF16, tag="nat")
                nat_sb[k] = natk
                nc.gpsimd.dma_start(
                    natk[:, : caps[k] * 512],
                    nat_d[:, nat_off[k] : nat_off[k + 1]],
                )
                pq = []  # software-pipeline: scores MMs trail p1 by 8 a-steps
                for j in range(nj):
                    wj = min(512, W - j * 512)
                    for a in range(AC):
                        ps = p1_psum.tile([128, 512], F32, tag="p1")
                        for hc in range(HC):
                            nc.tensor.matmul(
                                ps[:, :wj],
                                wt_sb[:, hc * A + a * 128 : hc * A + (a + 1) * 128],
                                xt[:, hc, j * 512 : j * 512 + wj],
                                start=(hc == 0),
                                stop=(hc == HC - 1),
                            )
                        if len(pq) >= 8:
                            pq.pop(0)()
                        pt = proj_pool.tile([128, 512], BF16, tag="projT")
                        nc.scalar.activation(
                            pt[:, :wj],
                            ps[:, :wj],
                            mybir.ActivationFunctionType.Tanh,
                            bias=pb_sb[:, a : a + 1],
                        )
                        # ctx col bwi is context's a-chunk, others zero, so only
                        # row bwi of the wave's scores psum accumulates slot k.
                        def pend(pt=pt, j=j, a=a, wj=wj):
                            nc.tensor.matmul(
                                scpss[w][j][:, :wj],
                                ctx_sb[:, (a * WB + bwi) * WB : (a * WB + bwi) * WB + wb],
                                pt[:, :wj],
                                start=(bwi == 0 and a == 0),
                                stop=(bwi == last_writer[w][j] and a == AC - 1),
                            )
                        pq.append(pend)
                for f in pq:
                    f()

            def finish_wave(w, b0, wb):
                Wmax = Wk[b0]
                njw = njs[b0]
                capm = caps[b0]
                mw = mask_sb[:wb, w * S : (w + 1) * S]
                # masked scores: scm = scores + additive mask, chunkwise
                scm = smx_pool.tile([wb, S], F32, tag="scm")
                pmax = smx_pool.tile([wb, 4], F32, tag="pmax")
                for j in range(njw):
                    wjj = min(512, Wmax - j * 512)
                    sl = slice(j * 512, j * 512 + wjj)
                    nc.vector.tensor_tensor(
                        out=scm[:, sl], in0=scpss[w][j][:, :wjj],
                        in1=mw[:, sl], op=mybir.AluOpType.add,
                    )
                    nc.vector.reduce_max(
                        pmax[:, j : j + 1], scm[:, sl], axis=mybir.AxisListType.X
                    )
                mx = smx_pool.tile([wb, 1], F32, tag="mx")
                nc.vector.reduce_max(
                    mx[:], pmax[:, :njw], axis=mybir.AxisListType.X, negate=True
                )
                ex = smx_pool.tile([wb, S], BF16, tag="ex")
                rs = smx_pool.tile([wb, 1], F32, tag="rs")
                nc.scalar.activation(
                    ex[:, :Wmax],
                    scm[:, :Wmax],
                    mybir.ActivationFunctionType.Exp,
                    bias=mx[:],
                    accum_out=rs[:],
                )
                rv = smx_pool.tile([wb, 1], F32, tag="rv")
                nc.vector.reciprocal(rv[:], rs[:])
                if wb > 1:
                    # move rv to partition 0 (tiny SBUF->SBUF DMA) so the
                    # final output copy can read it as a per-partition scale
                    rvT = smx_pool.tile([1, WB], F32, tag="rvT")
                    nc.gpsimd.dma_start(rvT[0:1, :wb], rv[:, 0:1])
                else:
                    rvT = rv  # single-row wave: already at partition 0
                # zero the ex tail beyond the wave width so the last
                # 128-chunk transposes clean zeros
                if capm * 128 > Wmax:
                    nc.vector.memset(ex[:, Wmax : capm * 128], 0.0)
                # transpose ex chunks [wb, 128] -> [128, wb] on PE (bf16);
                # column stride padded to 2 so bf16 PSUM offsets stay 4B-aligned
                cs = wb if wb % 2 == 0 else wb + 1
                for g in range((capm + 3) // 4):
                    na = min(4, capm - 4 * g)
                    aps = at_psum.tile([128, 4 * WB], BF16, tag="atps")
                    for jj in range(na):
                        ch = 4 * g + jj
                        nc.tensor.transpose(
                            aps[:, jj * cs : jj * cs + wb],
                            ex[:, ch * 128 : (ch + 1) * 128],
                            ident[:wb, :wb],
                        )
                    att_sb = attT_pool.tile([128, 4 * WB], BF16, tag="attT")
                    nc.vector.tensor_copy(
                        att_sb[:, : na * cs].rearrange(
                            "p (n c) -> p n c", c=cs
                        )[:, :, :wb],
                        aps[:, : na * cs].rearrange(
                            "p (n c) -> p n c", c=cs
                        )[:, :, :wb],
                    )
                    attT[(w, g)] = att_sb
                # phase 3: out[k] = (sum_s ex[s] * x[s, :]) * rv
                # smallest slot first so its output DMA lands earliest
                for bwi in reversed(range(wb)):
                    k = b0 + bwi
                    # single-row waves borrow a scores-ring bank (free by
                    # now) so the tail p3 never waits on the out-psum recycle
                    pool = sc_psum if wb == 1 else out_psum
                    tag = "scps" if wb == 1 else "ops"
                    ops = pool.tile([1, H], F32, tag=tag, name="ops")
                    for n in range(caps[k]):
                        col = (n % 4) * cs + bwi
                        nc.tensor.matmul(
                            ops[:],
                            attT[(w, n // 4)][:, col : col + 1],
                            nat_sb[k][:, n * 512 : (n + 1) * 512],
                            start=(n == 0),
                            stop=(n == caps[k] - 1),
                        )
                    os_b = os_pool.tile([1, H], F32, tag="os")
                    if wb == 1 and w % 2 == 0:
                        # alternate engines so consecutive single-slot waves'
                        # output scales run in parallel at the kernel tail
                        nc.scalar.activation(
                            os_b[:], ops[:],
                            mybir.ActivationFunctionType.Copy,
                            scale=rvT[0:1, 0:1],
                        )
                    else:
                        nc.vector.tensor_scalar_mul(
                            os_b[:], ops[:], rvT[0:1, bwi : bwi + 1]
                        )
                    (nc.sync if (wb == 1 and w % 2 == 0) else nc.scalar
                     ).dma_start(out_d[k : k + 1, :], os_b[:])

            for _rep in range(repeat):
                scpss.clear()
                attT.clear()
                fired = 0
                for w, (b0, wb) in enumerate(WAVE_SPANS):
                    scpss[w] = [
                        sc_psum.tile([wb, 512], F32, tag="scps", name="scps")
                        for _j in range(njs[b0])
                    ]
                    for bwi in range(wb):
                        trace_slot(w, b0, wb, bwi)
                        # fire pending finish_waves, but keep the last two
                        # tiny slots' p1 ahead of wave-1's finish so the
                        # final softmax chains start as early as possible
                        if bwi == 0 and w > 0 and w < 2:
                            finish_wave(fired, *WAVE_SPANS[fired])
                            fired += 1
                while fired < len(WAVE_SPANS):
                    finish_wave(fired, *WAVE_SPANS[fired])
                    fired += 1

    nc.finalize()
    return nc


_NC_CACHE = {}


def get_nc(caps, widths, repeat: int = 1) -> bass.Bass:
    key = (tuple(caps), tuple(widths), repeat)
    if key not in _NC_CACHE:
        _NC_CACHE[key] = build_nc(caps, widths, repeat=repeat)
    return _NC_CACHE[key]


def plan(lens):
    """Sort batches by length desc; slot k = ranks [8k, 8k+8), one per core.

    Window maxima of the descending sort minimize the summed per-slot caps
    (both the 20*L p1/scores term and the 512*ceil(L/128) p3 term are
    monotone in L, so length-sorting is optimal for the combined cost).
    """
    lens = np.asarray(lens).reshape(B).astype(np.int64)
    order = np.argsort(-lens, kind="stable")
    assign = order.reshape(BPC, NCORES)          # [slot, core] -> batch
    widths = [int(lens[assign[k, 0]]) for k in range(BPC)]
    caps = [(w + 127) // 128 for w in widths]
    return caps, widths, assign


def make_in_maps(nn_outs, batch_lens, context, proj_w, proj_b, caps, widths,
                 assign):
    x_bf = np.asarray(nn_outs, dtype=np.float32).astype(ml_dtypes.bfloat16)
    lens = np.asarray(batch_lens).reshape(B).astype(np.int64)
    wt = np.ascontiguousarray(np.asarray(proj_w, np.float32).T)  # [H, A]
    # wt_sb[p, c*A + a] = wt[128c + p, a]
    wt_host = np.ascontiguousarray(
        wt.reshape(HC, 128, A).transpose(1, 0, 2).reshape(128, HC * A)
    ).astype(ml_dtypes.bfloat16)
    ctx_c = np.asarray(context, np.float32).reshape(AC, 128)
    ctx_host = np.zeros((128, AC, WB, WB), np.float32)
    for a in range(AC):
        for bw in range(WB):
            ctx_host[:, a, bw, bw] = ctx_c[a]
    ctx_host = np.ascontiguousarray(
        ctx_host.reshape(128, AC * WB * WB)
    ).astype(ml_dtypes.bfloat16)
    pb_host = np.ascontiguousarray(
        np.asarray(proj_b, np.float32).reshape(AC, 128).T
    )
    iota = np.arange(S)[None, :]

    xt_w = sum(4 * w for w in widths)
    nat_w = sum(c * 512 for c in caps)
    in_maps = []
    for c in range(NCORES):
        xt_all = np.empty((128, xt_w), ml_dtypes.bfloat16)
        nat_all = np.empty((128, nat_w), ml_dtypes.bfloat16)
        mask = np.zeros((WB, len(WAVE_SPANS) * S), ml_dtypes.bfloat16)
        xo = no = 0
        for k in range(BPC):
            b = assign[k, c]
            W = widths[k]
            xa = x_bf[b, :W, :]                          # [W, H]
            xt_all[:, xo : xo + 4 * W] = (
                xa.T.reshape(HC, 128, W).transpose(1, 0, 2).reshape(128, 4 * W)
            )
            xo += 4 * W
            Wc = caps[k] * 128
            nat_all[:, no : no + caps[k] * 512] = (
                x_bf[b, :Wc, :].reshape(caps[k], 128, 512).transpose(1, 0, 2)
                .reshape(128, caps[k] * 512)
            )
            no += caps[k] * 512
        for w, (b0, wb) in enumerate(WAVE_SPANS):
            for bwi in range(wb):
                b = assign[b0 + bwi, c]
                mask[bwi, w * S : (w + 1) * S] = np.where(
                    iota[0] < lens[b], 0.0, -30000.0
                )
        in_maps.append(
            {
                "xt": xt_all,
                "nat": nat_all,
                "wt": wt_host,
                "ctx": ctx_host,
                "pb": pb_host,
                "mask": mask,
            }
        )
    return in_maps


def run(nn_outs, batch_lens, context, proj_w, proj_b, trace=False, repeat=1,
        **trace_kw):
    from concourse.bass_utils import run_bass_kernel_spmd

    caps, widths, assign = plan(batch_lens)
    nc = get_nc(caps, widths, repeat=repeat)
    in_maps = make_in_maps(
        nn_outs, batch_lens, context, proj_w, proj_b, caps, widths, assign
    )
    res = run_bass_kernel_spmd(
        nc, in_maps, list(range(NCORES)), trace=trace, **trace_kw
    )
    out = np.empty((B, H), np.float32)
    for c in range(NCORES):
        out[assign[:, c]] = res.results[c]["out"]
    return out, res


def kernel(nn_outs, batch_lens, context, proj_w, proj_b):
    out, _ = run(nn_outs, batch_lens, context, proj_w, proj_b, trace=False)
    return out
